# revision 5
# baseline (speedup 1.0000x reference)
"""LurieNet-k Trainium2 kernel, block-Picard formulation.

Per-step instruction overheads (ACT tanh ~320ns, DVE ~260-320ns) wall the
baseline per-step recurrence at ~770ns/step.  This kernel processes K=16
steps per block:
  x~_s   = A~^{s+1} x_base + H_s th1 + c_s          (guess th == th1)
  y_s    = C x~_{s-1} + by ;  th_s = tanh(y_s)      (wide quarter tanh)
  Delta  = (A~^K - I) x_base + sum_i G_i th_{K-1-i} + c_{K-1}
  x_base' = x_base + Delta                          (exact fp32 carry)
with A~ = I + 0.01A, G_i = A~^i (0.01B), H_s = sum_{i<=s} G_i, all
precomputed on device.  The constant-th guess contributes O(1e-3) output
error (validated vs reference: rel 2.5e-3); the identity part of the
carry only ever flows through fp32 (D = A~^K - I is applied in f32r and
added to x_base in fp32), so errors do not compound across blocks.

The trajectory is emitted as bf16 via PE pair-transposes into PSUM and
DMA'd straight from PSUM into two DRAM tensors (even/odd timesteps);
the host interleaves and upcasts.
"""

import sys

for _p in ("/opt/trn_rl_repo",):
    if _p not in sys.path:
        sys.path.insert(0, _p)

import numpy as np

import concourse.bass as bass
import concourse.mybir as mybir
import concourse.tile as tile
from concourse import bacc
from concourse import bass_isa
from concourse.bass import ds
from concourse.bass_utils import run_bass_kernel_spmd
from concourse.masks import make_identity, make_upper_triangular

F32 = mybir.dt.float32
F32R = mybir.dt.float32r
BF16 = mybir.dt.bfloat16
ALU = mybir.AluOpType
ACTF = mybir.ActivationFunctionType
AXIS = mybir.AxisListType

N = 128          # state dim
TMAX = 512       # time steps (including t=0)
BS = 512         # global batch
NCORES = 8
BSH = BS // NCORES   # 64 batch columns per core
STEP = 0.01
KTOP = 4
KB = 32          # block length (time steps per block)
NBLK = 16        # 15 full blocks + last block of 31 steps
KLAST = 31

EXPM_SCAL = 3
EXPM_TERMS = 4

PARAM_NAMES = [
    "ZA_Y", "ZA_U", "ZA_G", "ZB_U", "ZB_V", "ZB_S", "ZC_U", "ZC_V", "ZC_S",
]


def build_program():
    nc = bacc.Bacc(
        "TRN2",
        target_bir_lowering=False,
        debug=False,
        enable_asserts=False,
        num_devices=NCORES,
    )

    x0 = nc.dram_tensor("x0", [N, BSH], F32, kind="ExternalInput")
    zs = {
        name: nc.dram_tensor(name, [N, N], F32, kind="ExternalInput")
        for name in PARAM_NAMES
    }
    bx_d = nc.dram_tensor("bx", [N, 1], F32, kind="ExternalInput")
    by_d = nc.dram_tensor("by", [N, 1], F32, kind="ExternalInput")
    # even timesteps t=0,2,..,510 and odd t=1,3,..,511, bf16
    out_e = nc.dram_tensor("out_e", [BSH, TMAX // 2, N], BF16,
                           kind="ExternalOutput")
    out_o = nc.dram_tensor("out_o", [BSH, TMAX // 2, N], BF16,
                           kind="ExternalOutput")

    with tile.TileContext(nc) as tc:
        with tc.tile_pool(name="const", bufs=1) as constp:
            ident = constp.tile([N, N], F32, tag="ident")
            make_identity(nc, ident[:])
            masku = constp.tile([N, N], F32, tag="masku")
            make_upper_triangular(nc, masku[:], val=1.0, diag=False)
            ident_r32 = constp.tile([N, N], F32R, tag="ident_r32")
            nc.vector.tensor_copy(ident_r32[:], ident[:])
            identb = constp.tile([N, N], BF16, tag="identb")
            nc.vector.tensor_copy(identb[:], ident[:])

            by_c = constp.tile([N, 1], F32, tag="by")
            nc.scalar.dma_start(out=by_c[:], in_=by_d[:])
            bx_c = constp.tile([N, 1], F32, tag="bxraw")
            nc.gpsimd.dma_start(out=bx_c[:], in_=bx_d[:])
            bxp_c = constp.tile([N, 1], F32, tag="bxp")
            nc.vector.tensor_scalar_mul(bxp_c[:], bx_c[:], STEP)
            bxp_r = constp.tile([N, BSH], F32R, tag="bxpr")
            zer64 = constp.tile([N, BSH], F32, tag="zer64")
            nc.vector.memset(zer64[:], 0.0)
            nc.vector.tensor_scalar(
                bxp_r[:], zer64[:], bx_c[:], STEP,
                op0=ALU.add, op1=ALU.mult)
            x0_c = constp.tile([N, BSH], F32, tag="x0c")
            nc.scalar.dma_start(out=x0_c[:], in_=x0[:])

            # wide precomputed-weight tiles (transposed forms for lhsT use)
            ATpb = constp.tile([N, (KB - 1) * N], BF16, tag="ATpb")
            Hb = constp.tile([N, (KB - 1) * N], BF16, tag="Hb")
            Gb = constp.tile([N, KB * N], BF16, tag="Gb")
            DT16 = constp.tile([N, N], F32, tag="DT16")
            DT15 = constp.tile([N, N], F32, tag="DT15")
            CTb = constp.tile([N, N], BF16, tag="CTb")
            CTf32 = constp.tile([N, N], F32, tag="CTf32")
            Cmat16 = constp.tile([KB, N], BF16, tag="Cmat16")
            c15row = constp.tile([1, N], BF16, tag="c15row")  # c_{KB-1}
            c14row = constp.tile([1, N], BF16, tag="c14row")
            ind16 = constp.tile([KB, (KB - 1) * BSH], BF16, tag="ind16")
            ones64 = constp.tile([1, BSH], BF16, tag="ones64")
            nc.vector.memset(ones64[:], 1.0)
            nc.vector.memset(ind16[:], 0.0)
            for s in range(KB - 1):
                nc.gpsimd.dma_start(out=ind16[s:s + 1, ds(s * BSH, BSH)],
                                    in_=ones64[:])

            # ------- setup: expm's + weight assembly -------
            with (
                tc.tile_pool(name="zbuf", bufs=1) as zp,
                tc.tile_pool(name="work", bufs=2) as wp,
                tc.tile_pool(name="eres", bufs=1) as ep,
                tc.tile_pool(name="small", bufs=1) as sp,
                tc.tile_pool(name="pow", bufs=1) as powp,
                tc.tile_pool(name="pss", bufs=4, space="PSUM") as psp,
            ):
                zt = {}
                _order = ["ZC_U", "ZC_V", "ZB_U", "ZB_V", "ZA_U",
                          "ZC_S", "ZB_S", "ZA_G", "ZA_Y"]
                _qs = {"ZC_U": nc.sync, "ZC_V": nc.sync, "ZB_U": nc.sync,
                       "ZB_V": nc.scalar, "ZA_U": nc.scalar,
                       "ZC_S": nc.sync, "ZB_S": nc.scalar,
                       "ZA_G": nc.gpsimd, "ZA_Y": nc.gpsimd}
                for name in _order:
                    zt[name] = zp.tile([N, N], F32, tag=name, name=f"z_{name}")
                    _qs[name].dma_start(out=zt[name][:], in_=zs[name][:])

                def expm_batch(specs):
                    """Interleaved expm(skew(Z))^T for all matrices at once."""
                    scal = 1.0 / (2.0 ** EXPM_SCAL)
                    negx = {}
                    t_cur = {}
                    tt_cur = {}
                    for z_tile, tag in specs:
                        us = wp.tile([N, N], F32R, tag="us_r", name=f"us_{tag}")
                        nc.vector.scalar_tensor_tensor(
                            us[:], z_tile[:], scal, masku[:],
                            op0=ALU.mult, op1=ALU.mult,
                        )
                        pst = psp.tile([N, N], F32R, tag="ps", bufs=4,
                                       name=f"pst_{tag}")
                        nc.tensor.transpose(pst[:], us[:], ident_r32[:])
                        nx = wp.tile([N, N], F32R, tag=f"negx_{tag}", bufs=1,
                                     name=f"negx_{tag}")
                        nc.vector.scalar_tensor_tensor(
                            nx[:], pst[:], 1.0, us[:],
                            op0=ALU.mult, op1=ALU.subtract,
                        )
                        negx[tag] = nx
                        t_cur[tag] = ident_r32
                        tt_cur[tag] = ident_r32
                    for j in range(EXPM_TERMS, 0, -1):
                        for _, tag in specs:
                            psa = psp.tile([N, N], F32, tag="ps", bufs=4)
                            nc.tensor.matmul(
                                psa[:], negx[tag][:], t_cur[tag][:],
                                start=True, stop=True,
                            )
                            t_new = wp.tile([N, N], F32R, tag=f"T_{tag}",
                                            bufs=2, name=f"T_{tag}")
                            nc.vector.scalar_tensor_tensor(
                                t_new[:], psa[:], 1.0 / j, ident_r32[:],
                                op0=ALU.mult, op1=ALU.add,
                            )
                            t_cur[tag] = t_new
                    for _, tag in specs:
                        pst = psp.tile([N, N], F32R, tag="ps", bufs=4,
                                       name=f"ptt_{tag}")
                        nc.tensor.transpose(pst[:], t_cur[tag][:], ident_r32[:])
                        tt_new = wp.tile([N, N], F32R, tag=f"TT_{tag}",
                                         bufs=2, name=f"TT_{tag}")
                        nc.scalar.copy(tt_new[:], pst[:])
                        tt_cur[tag] = tt_new
                    for _ in range(EXPM_SCAL):
                        for _, tag in specs:
                            psa = psp.tile([N, N], F32, tag="ps", bufs=4)
                            psb = psp.tile([N, N], F32, tag="ps", bufs=4)
                            nc.tensor.matmul(
                                psa[:], tt_cur[tag][:], t_cur[tag][:],
                                start=True, stop=True,
                            )
                            nc.tensor.matmul(
                                psb[:], t_cur[tag][:], tt_cur[tag][:],
                                start=True, stop=True,
                            )
                            t_new = wp.tile([N, N], F32R, tag=f"T_{tag}",
                                            bufs=2, name=f"T_{tag}")
                            tt_new = wp.tile([N, N], F32R, tag=f"TT_{tag}",
                                             bufs=2, name=f"TT_{tag}")
                            nc.vector.tensor_copy(t_new[:], psa[:])
                            nc.scalar.copy(tt_new[:], psb[:])
                            t_cur[tag], tt_cur[tag] = t_new, tt_new
                    return tt_cur

                eres = expm_batch([
                    (zt["ZC_U"], "UCT"), (zt["ZC_V"], "VCT"),
                    (zt["ZB_U"], "UBT"), (zt["ZB_V"], "VBT"),
                    (zt["ZA_U"], "UAT"),
                ])
                uct, vct = eres["UCT"], eres["VCT"]
                ubt, vbt = eres["UBT"], eres["VBT"]
                uat = eres["UAT"]

                def absdiag_col(z_tile, tag):
                    tmp = wp.tile([N, N], F32, tag="us")
                    nc.vector.tensor_mul(tmp[:], z_tile[:], ident[:])
                    col = sp.tile([N, 1], F32, tag=tag, name=f"col_{tag}")
                    nc.vector.tensor_reduce(
                        col[:], tmp[:], AXIS.X, ALU.add,
                        apply_absolute_value=True
                    )
                    return col

                dc_col = absdiag_col(zt["ZC_S"], "dc")
                db_col = absdiag_col(zt["ZB_S"], "db")
                ga_col = absdiag_col(zt["ZA_G"], "ga")

                # top-4: alpha = sqrt(sum_i (b_i c_i)^2)
                bwork = sp.tile([N, 1], F32, tag="bwork")
                cwork = sp.tile([N, 1], F32, tag="cwork")
                nc.vector.tensor_copy(bwork[:], db_col[:])
                nc.vector.tensor_copy(cwork[:], dc_col[:])
                acc = sp.tile([N, 1], F32, tag="acc")
                nc.vector.memset(acc[:], 0.0)
                bmax = sp.tile([N, 1], F32, tag="bmax")
                cmax = sp.tile([N, 1], F32, tag="cmax")
                prod = sp.tile([N, 1], F32, tag="prod")
                gmask = sp.tile([N, 1], F32, tag="gmask")
                tdrop = sp.tile([N, 1], F32, tag="tdrop")
                for i in range(KTOP):
                    nc.gpsimd.partition_all_reduce(
                        bmax[:], bwork[:], N, bass_isa.ReduceOp.max
                    )
                    nc.gpsimd.partition_all_reduce(
                        cmax[:], cwork[:], N, bass_isa.ReduceOp.max
                    )
                    nc.vector.tensor_mul(prod[:], bmax[:], cmax[:])
                    nc.vector.tensor_mul(prod[:], prod[:], prod[:])
                    nc.vector.tensor_add(acc[:], acc[:], prod[:])
                    if i < KTOP - 1:
                        nc.vector.tensor_single_scalar(
                            gmask[:], bwork[:], bmax[:], ALU.is_ge
                        )
                        nc.vector.tensor_mul(tdrop[:], bwork[:], gmask[:])
                        nc.vector.tensor_sub(bwork[:], bwork[:], tdrop[:])
                        nc.vector.tensor_single_scalar(
                            gmask[:], cwork[:], cmax[:], ALU.is_ge
                        )
                        nc.vector.tensor_mul(tdrop[:], cwork[:], gmask[:])
                        nc.vector.tensor_sub(cwork[:], cwork[:], tdrop[:])
                alpha = sp.tile([N, 1], F32, tag="alpha")
                nc.scalar.activation(alpha[:], acc[:], ACTF.Sqrt)

                sa05 = sp.tile([N, 1], F32, tag="sa05")
                nc.vector.tensor_scalar(
                    sa05[:], ga_col[:], alpha[:], -0.5,
                    op0=ALU.add, op1=ALU.mult
                )
                sb01 = sp.tile([N, 1], F32, tag="sb01")
                nc.vector.tensor_scalar_mul(sb01[:], db_col[:], STEP)

                # C^T = VC @ (SC @ UC^T)
                p1 = wp.tile([N, N], F32R, tag="us_r", name="p1")
                nc.vector.tensor_scalar_mul(p1[:], uct[:], dc_col[:])
                psa = psp.tile([N, N], F32, tag="ps", bufs=4)
                nc.tensor.matmul(psa[:], vct[:], p1[:], start=True, stop=True)
                nc.vector.tensor_copy(CTf32[:], psa[:])
                nc.scalar.copy(CTb[:], psa[:])

                # untransposed 0.01 B = UB @ (0.01 SB @ VB^T)
                p2b = wp.tile([N, N], F32R, tag="us_r", name="p2b")
                nc.vector.tensor_scalar_mul(p2b[:], vbt[:], sb01[:])
                psb2 = psp.tile([N, N], F32, tag="ps", bufs=4)
                nc.tensor.matmul(psb2[:], ubt[:], p2b[:], start=True, stop=True)
                bp_un = ep.tile([N, N], F32, tag="Bpun")
                nc.vector.tensor_copy(bp_un[:], psb2[:])

                # M = UA @ (sa05 * UA^T) = 0.5*UA SA UA^T (symmetric)
                p3 = wp.tile([N, N], F32R, tag="us_r", name="p3")
                nc.vector.tensor_scalar_mul(p3[:], uat[:], sa05[:])
                psm = psp.tile([N, N], F32, tag="ps", bufs=4)
                nc.tensor.matmul(psm[:], uat[:], p3[:], start=True, stop=True)
                # YA = Uy - Uy^T; q2 = -0.005*YA
                uy = wp.tile([N, N], F32, tag="us")
                nc.vector.tensor_mul(uy[:], zt["ZA_Y"][:], masku[:])
                pst2 = psp.tile([N, N], F32, tag="ps", bufs=4)
                nc.tensor.transpose(pst2[:], uy[:], ident[:])
                nc.vector.tensor_scalar_mul(uy[:], uy[:], 0.5 * STEP)
                q2 = wp.tile([N, N], F32, tag="T")
                nc.vector.scalar_tensor_tensor(
                    q2[:], pst2[:], 0.5 * STEP, uy[:],
                    op0=ALU.mult, op1=ALU.subtract
                )
                # A~^T = I + (0.01 A)^T ; A~ un-transposed = I + 0.01 A
                ATp1 = powp.tile([N, N], F32, tag="ATp1", name="ATp1")
                a01T = wp.tile([N, N], F32, tag="a01T", bufs=1)
                nc.vector.scalar_tensor_tensor(
                    a01T[:], psm[:], STEP, q2[:], op0=ALU.mult, op1=ALU.add
                )
                nc.vector.tensor_add(ATp1[:], a01T[:], ident[:])
                a01_un = wp.tile([N, N], F32, tag="a01un", bufs=1)
                nc.vector.scalar_tensor_tensor(
                    a01_un[:], psm[:], STEP, q2[:],
                    op0=ALU.mult, op1=ALU.subtract
                )
                Aun = ep.tile([N, N], F32, tag="Aun")
                nc.vector.tensor_add(Aun[:], a01_un[:], ident[:])

                # ---- power chain ATp[i] = A~T^i, log-depth ----
                # fp32 only on the squaring path (feeds D = A~^K - I);
                # branch powers f32r (feed bf16 ATpb/G/H/c only).
                ATp = [None] * (KB + 1)      # f32r (or fp32 for 2^k)
                ATpr = [None] * (KB + 1)     # f32r view for branch rhs
                ATp[1] = ATp1
                ATp1r = powp.tile([N, N], F32R, tag="ATp1r", name="ATp1r")
                nc.vector.tensor_copy(ATp1r[:], ATp1[:])
                ATpr[1] = ATp1r
                Aunp = {1: Aun}
                Aunr = {}
                Aun1r = powp.tile([N, N], F32R, tag="Aun1r", name="Aun1r")
                nc.scalar.copy(Aun1r[:], Aun[:])
                Aunr[1] = Aun1r

                def _mk_pow(i, lhs_un, rhs_t):
                    # f32r branch power
                    psq = psp.tile([N, N], F32, tag="ps", bufs=4)
                    nc.tensor.matmul(psq[:], lhs_un[:], rhs_t[:],
                                     start=True, stop=True)
                    t_ = powp.tile([N, N], F32R, tag=f"ATp{i}", name=f"ATp{i}")
                    if i % 2 == 0:
                        nc.vector.tensor_copy(t_[:], psq[:])
                    else:
                        nc.scalar.copy(t_[:], psq[:])
                    ATp[i] = t_
                    ATpr[i] = t_
                    if i <= KB - 1:
                        nc.scalar.copy(ATpb[:, ds((i - 1) * N, N)], psq[:])

                for lvl in (1, 2, 4, 8, 16):
                    for i in range(lvl + 1, 2 * lvl, 1):
                        _mk_pow(i, Aunr[lvl], ATpr[i - lvl])
                    # squaring (fp32): power 2*lvl
                    psq = psp.tile([N, N], F32, tag="ps", bufs=4)
                    nc.tensor.matmul(psq[:], Aunp[lvl][:], ATp[lvl][:],
                                     start=True, stop=True)
                    t_ = powp.tile([N, N], F32, tag=f"ATp{2 * lvl}",
                                   name=f"ATp{2 * lvl}")
                    nc.vector.tensor_copy(t_[:], psq[:])
                    ATp[2 * lvl] = t_
                    tr = powp.tile([N, N], F32R, tag=f"ATpr{2 * lvl}",
                                   name=f"ATpr{2 * lvl}")
                    nc.scalar.copy(tr[:], psq[:])
                    ATpr[2 * lvl] = tr
                    if 2 * lvl <= KB - 1:
                        nc.scalar.copy(ATpb[:, ds((2 * lvl - 1) * N, N)],
                                       psq[:])
                    if 2 * lvl < KB:
                        psu = psp.tile([N, N], F32, tag="ps", bufs=4)
                        nc.tensor.matmul(psu[:], ATp[lvl][:], Aunp[lvl][:],
                                         start=True, stop=True)
                        u_ = powp.tile([N, N], F32, tag=f"Aun{2 * lvl}",
                                       name=f"Aun{2 * lvl}")
                        nc.vector.tensor_copy(u_[:], psu[:])
                        Aunp[2 * lvl] = u_
                        ur = powp.tile([N, N], F32R, tag=f"Aunr{2 * lvl}",
                                       name=f"Aunr{2 * lvl}")
                        nc.scalar.copy(ur[:], psu[:])
                        Aunr[2 * lvl] = ur
                nc.gpsimd.tensor_copy(ATpb[:, ds(0, N)], ATp1[:])

                # D^T = A~T^K - I
                nc.vector.tensor_sub(DT16[:], ATp[KB][:], ident[:])
                nc.vector.tensor_sub(DT15[:], ATp[KB - 1][:], ident[:])

                # ---- G_i^T = (0.01B)^T A~T^i, i=0..15 ----
                bp_un_r = ep.tile([N, N], F32R, tag="Bpunr")
                nc.vector.tensor_copy(bp_un_r[:], bp_un[:])
                for i in range(KB):
                    psg = psp.tile([N, N], F32, tag="ps", bufs=4,
                                   name=f"G{i}")
                    rhs = ident_r32 if i == 0 else ATpr[i]
                    nc.tensor.matmul(psg[:], bp_un_r[:], rhs[:],
                                     start=True, stop=True)
                    if i % 2 == 0:
                        nc.scalar.copy(Gb[:, ds(i * N, N)], psg[:])
                    else:
                        nc.vector.tensor_copy(Gb[:, ds(i * N, N)], psg[:])

                # ---- H_s^T = sum_{i<=s} G_i^T (pairing) ----
                nc.gpsimd.tensor_copy(Hb[:, ds(0, N)], Gb[:, ds(0, N)])
                nc.gpsimd.tensor_add(Hb[:, ds(N, N)], Gb[:, ds(0, N)],
                                     Gb[:, ds(N, N)])
                npair = KB // 2 - 1
                gpair = ep.tile([N, npair * N], BF16, tag="gpair")
                for k in range(1, npair):
                    nc.vector.tensor_add(
                        gpair[:, ds(k * N, N)],
                        Gb[:, ds(2 * k * N, N)],
                        Gb[:, ds((2 * k + 1) * N, N)])
                for k in range(1, KB // 2):
                    # serial chain on odd H; even H branches on Pool
                    nc.gpsimd.tensor_add(
                        Hb[:, ds(2 * k * N, N)],
                        Hb[:, ds((2 * k - 1) * N, N)],
                        Gb[:, ds(2 * k * N, N)])
                    if 2 * k + 1 <= KB - 2:
                        nc.vector.tensor_add(
                            Hb[:, ds((2 * k + 1) * N, N)],
                            Hb[:, ds((2 * k - 1) * N, N)],
                            gpair[:, ds(k * N, N)])

                # ---- c columns: p_s = A~^s bx' then prefix sum ----
                pcols = sp.tile([N, KB], F32, tag="pcols")
                for s in range(KB):
                    lhs = ident_r32 if s == 0 else ATpr[s]
                    pc = psp.tile([N, N], F32, tag="ps", bufs=4,
                                  name=f"pc{s}")
                    nc.tensor.matmul(pc[:, 0:BSH], lhs[:],
                                     bxp_r[:], start=True, stop=True)
                    eng = nc.vector if s % 2 == 0 else nc.scalar
                    if s % 2 == 0:
                        nc.vector.tensor_copy(pcols[:, s:s + 1], pc[:, 0:1])
                    else:
                        nc.scalar.copy(pcols[:, s:s + 1], pc[:, 0:1])
                ccols = sp.tile([N, KB], F32, tag="ccols")
                onesc = sp.tile([N, KB], F32, tag="onesc")
                nc.vector.memset(onesc[:], 1.0)
                nc.vector.tensor_tensor_scan(
                    ccols[:], onesc[:], pcols[:], 0.0,
                    op0=ALU.mult, op1=ALU.add,
                )
                # transpose -> Cmat16 [16, 128] bf16
                cpst = psp.tile([KB, N], F32, tag="cs2", bufs=1, name="cpst")
                nc.tensor.transpose(cpst[:], ccols[:], ident[:])
                nc.scalar.copy(Cmat16[:], cpst[:])
                c15ps = psp.tile([1, N], F32, tag="cs3", bufs=2, name="c15ps")
                nc.tensor.transpose(c15ps[:], ccols[:, KB - 1:KB], ident[:])
                nc.scalar.copy(c15row[:], c15ps[:])
                c14ps = psp.tile([1, N], F32, tag="cs3", bufs=2, name="c14ps")
                nc.tensor.transpose(c14ps[:], ccols[:, KB - 2:KB - 1], ident[:])
                nc.scalar.copy(c14row[:], c14ps[:])

            # ------- block loop -------
            with (
                tc.tile_pool(name="xsb", bufs=2) as xsbp,
                tc.tile_pool(name="stg", bufs=2) as stgp,
                tc.tile_pool(name="thb", bufs=2) as thp,
                tc.tile_pool(name="base", bufs=2) as basep,
                tc.tile_pool(name="xps", bufs=1, space="PSUM") as xpsp,
                tc.tile_pool(name="psy", bufs=2, space="PSUM") as psyp,
                tc.tile_pool(name="dps", bufs=1, space="PSUM") as dpsp,
                tc.tile_pool(name="trp", bufs=1, space="PSUM") as trpp,
            ):
                # init: th_init = tanh(C x0 + by), base = x0
                psy0 = psyp.tile([N, BSH], F32, tag="psyq", name="psy0")
                nc.tensor.matmul(psy0[:], CTf32[:], x0_c[:],
                                 start=True, stop=True)
                th_init = thp.tile([N, KB * BSH], BF16, tag="th",
                                   name="th_init")
                nc.scalar.activation(
                    th_init[:, ds((KB - 1) * BSH, BSH)], psy0[:],
                    ACTF.Tanh, bias=by_c[:], scale=1.0
                )
                base_cur = basep.tile([N, BSH], F32, tag="base",
                                      name="base0")
                nc.vector.tensor_copy(base_cur[:], x0_c[:])
                xsb_cur = xsbp.tile([N, KB * BSH], BF16, tag="xsb",
                                    name="xsb0")
                nc.scalar.copy(xsb_cur[:, ds(0, BSH)], x0_c[:])
                th_prev = th_init

                for j in range(NBLK):
                    kb = KB if j < NBLK - 1 else KLAST
                    ncols = (kb - 1) * BSH
                    th1 = th_prev[:, ds((KB - 1) * BSH, BSH)]
                    base_b = xsb_cur[:, ds(0, BSH)]
                    xq = xpsp.tile([N, (KB - 1) * BSH], F32, tag="xq")
                    # consts first (opens accumulation, no deps)
                    for off in range(0, ncols, 512):
                        cwid = min(512, ncols - off)
                        nc.tensor.matmul(
                            xq[:, ds(off, cwid)], Cmat16[:],
                            ind16[:, ds(off, cwid)],
                            start=True, stop=False, skip_group_check=True)
                    for s in range(kb - 1):
                        nc.tensor.matmul(
                            xq[:, ds(s * BSH, BSH)],
                            ATpb[:, ds(s * N, N)], base_b,
                            start=False, stop=False, skip_group_check=True)
                        nc.tensor.matmul(
                            xq[:, ds(s * BSH, BSH)],
                            Hb[:, ds(s * N, N)], th1,
                            start=False, stop=True, skip_group_check=True)
                    # quarters: cast -> psy -> tanh
                    th_cur = thp.tile([N, KB * BSH], BF16, tag="th")
                    nq = KB // 4
                    qbounds = [4 * i for i in range(nq)] + [kb]
                    for q in range(nq):
                        s0, s1 = qbounds[q], qbounds[q + 1]
                        w = (s1 - s0) * BSH
                        cw = min(s1, kb - 1) - s0
                        if cw > 0:
                            nc.vector.tensor_copy(
                                xsb_cur[:, ds((1 + s0) * BSH, cw * BSH)],
                                xq[:, ds(s0 * BSH, cw * BSH)],
                            )
                        psyq = psyp.tile([N, KB // 4 * BSH], F32, tag="psyq")
                        nc.tensor.matmul(
                            psyq[:, ds(0, w)], CTb[:],
                            xsb_cur[:, ds(s0 * BSH, w)],
                            start=True, stop=True)
                        nc.scalar.activation(
                            th_cur[:, ds(s0 * BSH, w)], psyq[:, ds(0, w)],
                            ACTF.Tanh, bias=by_c[:], scale=1.0
                        )
                    # refine: Delta = D x_base + sum G_i th_{kb-1-i} + c_{kb-1}
                    dq = dpsp.tile([N, BSH], F32, tag="dq")
                    nc.tensor.matmul(dq[:], DT16[:] if kb == KB else DT15[:],
                                     base_cur[:], start=True, stop=False,
                                     skip_group_check=True)
                    nc.tensor.matmul(dq[:], (c15row if kb == KB else c14row)[:],
                                     ones64[:], start=False, stop=False,
                                     skip_group_check=True)
                    for s in range(kb - 1, -1, -1):
                        i = kb - 1 - s
                        nc.tensor.matmul(
                            dq[:], Gb[:, ds(i * N, N)],
                            th_cur[:, ds(s * BSH, BSH)],
                            start=False, stop=(s == 0),
                            skip_group_check=True)
                    base_new = basep.tile([N, BSH], F32, tag="base")
                    nc.vector.tensor_add(base_new[:], dq[:], base_cur[:])
                    if j < NBLK - 1:
                        xsb_new = xsbp.tile([N, KB * BSH], BF16, tag="xsb")
                        nc.vector.tensor_add(
                            xsb_new[:, ds(0, BSH)], dq[:], base_cur[:])
                    else:
                        # last block: x_511 goes into this block's col 15
                        xsb_new = None
                        nc.vector.tensor_add(
                            xsb_cur[:, ds((KB - 1) * BSH, BSH)],
                            dq[:], base_cur[:])
                    # output transposes: pairs (2i, 2i+1) of xsb_cur cols
                    t0 = 1 + KB * j
                    for h in range(2):
                        trp = trpp.tile([N, 8 * N], BF16, tag="trp")
                        for i in range(8):
                            nc.tensor.transpose(
                                trp[:, ds(i * N, N)],
                                xsb_cur[:, ds((16 * h + 2 * i) * BSH,
                                              2 * BSH)],
                                identb[:],
                            )
                        stg = stgp.tile([N, 8 * N], BF16, tag="stg")
                        nc.vector.tensor_copy(stg[:], trp[:])
                        nc.sync.dma_start(
                            out=out_e[:, ds((t0 - 1) // 2 + 8 * h, 8),
                                      :].rearrange("b i n -> b (i n)"),
                            in_=stg[0:BSH, :],
                        )
                        nc.sync.dma_start(
                            out=out_o[:, ds((t0 - 1) // 2 + 8 * h, 8),
                                      :].rearrange("b i n -> b (i n)"),
                            in_=stg[BSH:2 * BSH, :],
                        )
                    base_cur = base_new
                    th_prev = th_cur
                    xsb_cur = xsb_new

    nc.compile()
    return nc


_CACHED = {}


def _get_program(*_args, **_kw):
    if "p" not in _CACHED:
        _CACHED["p"] = build_program()
    return _CACHED["p"]


def make_in_maps(inputs):
    X0 = np.ascontiguousarray(np.asarray(inputs["X0"], dtype=np.float32))
    base = {
        name: np.ascontiguousarray(np.asarray(inputs[name], dtype=np.float32))
        for name in PARAM_NAMES
    }
    base["bx"] = np.ascontiguousarray(
        np.asarray(inputs["bx"], dtype=np.float32).reshape(N, 1)
    )
    base["by"] = np.ascontiguousarray(
        np.asarray(inputs["by"], dtype=np.float32).reshape(N, 1)
    )
    in_maps = []
    for c in range(NCORES):
        m = dict(base)
        m["x0"] = np.ascontiguousarray(X0[c * BSH:(c + 1) * BSH].T)
        in_maps.append(m)
    return in_maps


def run_spmd(inputs, *_args, trace=False, tmpdir=None, **_kw):
    nc = _get_program()
    in_maps = make_in_maps(inputs)
    res = run_bass_kernel_spmd(
        nc, in_maps, list(range(NCORES)), trace=trace, tmpdir=tmpdir
    )
    outs = []
    for c in range(NCORES):
        ev = np.asarray(res.results[c]["out_e"]).astype(np.float32)
        od = np.asarray(res.results[c]["out_o"]).astype(np.float32)
        full = np.empty((BSH, TMAX, N), dtype=np.float32)
        full[:, 0::2, :] = ev
        full[:, 1::2, :] = od
        outs.append(full)
    return np.concatenate(outs, axis=0), res


def kernel(**inputs):
    full, _ = run_spmd(inputs)
    return full


# revision 6
# speedup vs baseline: 1.2119x; 1.2119x over previous
"""LurieNet-k Trainium2 kernel, block-Picard formulation.

Per-step instruction overheads (ACT tanh ~320ns, DVE ~260-320ns) wall the
baseline per-step recurrence at ~770ns/step.  This kernel processes K=16
steps per block:
  x~_s   = A~^{s+1} x_base + H_s th1 + c_s          (guess th == th1)
  y_s    = C x~_{s-1} + by ;  th_s = tanh(y_s)      (wide quarter tanh)
  Delta  = (A~^K - I) x_base + sum_i G_i th_{K-1-i} + c_{K-1}
  x_base' = x_base + Delta                          (exact fp32 carry)
with A~ = I + 0.01A, G_i = A~^i (0.01B), H_s = sum_{i<=s} G_i, all
precomputed on device.  The constant-th guess contributes O(1e-3) output
error (validated vs reference: rel 2.5e-3); the identity part of the
carry only ever flows through fp32 (D = A~^K - I is applied in f32r and
added to x_base in fp32), so errors do not compound across blocks.

The trajectory is emitted as bf16 via PE pair-transposes into PSUM and
DMA'd straight from PSUM into two DRAM tensors (even/odd timesteps);
the host interleaves and upcasts.
"""

import sys

for _p in ("/opt/trn_rl_repo",):
    if _p not in sys.path:
        sys.path.insert(0, _p)

import numpy as np

import concourse.bass as bass
import concourse.mybir as mybir
import concourse.tile as tile
from concourse import bacc
from concourse import bass_isa
from concourse.bass import ds
from concourse.bass_utils import run_bass_kernel_spmd
from concourse.masks import make_identity, make_upper_triangular

F32 = mybir.dt.float32
F32R = mybir.dt.float32r
BF16 = mybir.dt.bfloat16
ALU = mybir.AluOpType
ACTF = mybir.ActivationFunctionType
AXIS = mybir.AxisListType

N = 128          # state dim
TMAX = 512       # time steps (including t=0)
BS = 512         # global batch
NCORES = 8
BSH = BS // NCORES   # 64 batch columns per core
STEP = 0.01
KTOP = 4
KB = 32          # block length (time steps per block)
NBLK = 16        # 15 full blocks + last block of 31 steps
KLAST = 31

EXPM_SCAL = 3
EXPM_TERMS = 4

PARAM_NAMES = [
    "ZA_Y", "ZA_U", "ZA_G", "ZB_U", "ZB_V", "ZB_S", "ZC_U", "ZC_V", "ZC_S",
]


def build_program():
    nc = bacc.Bacc(
        "TRN2",
        target_bir_lowering=False,
        debug=False,
        enable_asserts=False,
        num_devices=NCORES,
    )

    x0 = nc.dram_tensor("x0", [N, BSH], F32, kind="ExternalInput")
    zs = {
        name: nc.dram_tensor(name, [N, N], F32, kind="ExternalInput")
        for name in PARAM_NAMES
    }
    bx_d = nc.dram_tensor("bx", [N, 1], F32, kind="ExternalInput")
    by_d = nc.dram_tensor("by", [N, 1], F32, kind="ExternalInput")
    # even timesteps t=0,2,..,510 and odd t=1,3,..,511, bf16
    out_e = nc.dram_tensor("out_e", [BSH, TMAX // 2, N], BF16,
                           kind="ExternalOutput")
    out_o = nc.dram_tensor("out_o", [BSH, TMAX // 2, N], BF16,
                           kind="ExternalOutput")

    with tile.TileContext(nc) as tc:
        with tc.tile_pool(name="const", bufs=1) as constp:
            ident = constp.tile([N, N], F32, tag="ident")
            make_identity(nc, ident[:])
            masku = constp.tile([N, N], F32, tag="masku")
            make_upper_triangular(nc, masku[:], val=1.0, diag=False)
            ident_r32 = constp.tile([N, N], F32R, tag="ident_r32")
            nc.vector.tensor_copy(ident_r32[:], ident[:])
            identb = constp.tile([N, N], BF16, tag="identb")
            nc.vector.tensor_copy(identb[:], ident[:])

            by_c = constp.tile([N, 1], F32, tag="by")
            nc.scalar.dma_start(out=by_c[:], in_=by_d[:])
            bx_c = constp.tile([N, 1], F32, tag="bxraw")
            nc.gpsimd.dma_start(out=bx_c[:], in_=bx_d[:])
            bxp_c = constp.tile([N, 1], F32, tag="bxp")
            nc.vector.tensor_scalar_mul(bxp_c[:], bx_c[:], STEP)
            bxp_r = constp.tile([N, BSH], F32, tag="bxpr")
            zer64 = constp.tile([N, BSH], F32, tag="zer64")
            nc.vector.memset(zer64[:], 0.0)
            nc.vector.tensor_scalar(
                bxp_r[:], zer64[:], bx_c[:], STEP,
                op0=ALU.add, op1=ALU.mult)
            x0_c = constp.tile([N, BSH], F32, tag="x0c")
            nc.scalar.dma_start(out=x0_c[:], in_=x0[:])

            # wide precomputed-weight tiles (transposed forms for lhsT use)
            ATpb = constp.tile([N, (KB - 1) * N], BF16, tag="ATpb")
            Hb = constp.tile([N, (KB - 1) * N], BF16, tag="Hb")
            Gb = constp.tile([N, KB * N], BF16, tag="Gb")
            DT16 = constp.tile([N, N], F32, tag="DT16")
            DT15 = constp.tile([N, N], F32, tag="DT15")
            CTb = constp.tile([N, N], BF16, tag="CTb")
            CTf32 = constp.tile([N, N], F32, tag="CTf32")
            Cmat16 = constp.tile([KB, N], BF16, tag="Cmat16")
            c15row = constp.tile([1, N], BF16, tag="c15row")  # c_{KB-1}
            c14row = constp.tile([1, N], BF16, tag="c14row")
            ind16 = constp.tile([KB, (KB - 1) * BSH], BF16, tag="ind16")
            ones64 = constp.tile([1, BSH], BF16, tag="ones64")
            nc.vector.memset(ones64[:], 1.0)
            nc.vector.memset(ind16[:], 0.0)
            for s in range(KB - 1):
                nc.gpsimd.dma_start(out=ind16[s:s + 1, ds(s * BSH, BSH)],
                                    in_=ones64[:])

            # ------- setup: expm's + weight assembly -------
            with (
                tc.tile_pool(name="zbuf", bufs=1) as zp,
                tc.tile_pool(name="work", bufs=2) as wp,
                tc.tile_pool(name="eres", bufs=1) as ep,
                tc.tile_pool(name="small", bufs=1) as sp,
                tc.tile_pool(name="pow", bufs=1) as powp,
                tc.tile_pool(name="pss", bufs=4, space="PSUM") as psp,
            ):
                zt = {}
                _order = ["ZC_U", "ZC_V", "ZB_U", "ZB_V", "ZA_U",
                          "ZC_S", "ZB_S", "ZA_G", "ZA_Y"]
                _qs = {"ZC_U": nc.sync, "ZC_V": nc.sync, "ZB_U": nc.sync,
                       "ZB_V": nc.scalar, "ZA_U": nc.scalar,
                       "ZC_S": nc.sync, "ZB_S": nc.scalar,
                       "ZA_G": nc.gpsimd, "ZA_Y": nc.gpsimd}
                for name in _order:
                    zt[name] = zp.tile([N, N], F32, tag=name, name=f"z_{name}")
                    _qs[name].dma_start(out=zt[name][:], in_=zs[name][:])

                def expm_batch(specs):
                    """Interleaved expm(skew(Z))^T for all matrices at once."""
                    scal = 1.0 / (2.0 ** EXPM_SCAL)
                    negx = {}
                    t_cur = {}
                    tt_cur = {}
                    for z_tile, tag in specs:
                        us = wp.tile([N, N], F32R, tag="us_r", name=f"us_{tag}")
                        nc.vector.scalar_tensor_tensor(
                            us[:], z_tile[:], scal, masku[:],
                            op0=ALU.mult, op1=ALU.mult,
                        )
                        pst = psp.tile([N, N], F32R, tag="ps", bufs=4,
                                       name=f"pst_{tag}")
                        nc.tensor.transpose(pst[:], us[:], ident_r32[:])
                        nx = wp.tile([N, N], F32R, tag=f"negx_{tag}", bufs=1,
                                     name=f"negx_{tag}")
                        nc.vector.scalar_tensor_tensor(
                            nx[:], pst[:], 1.0, us[:],
                            op0=ALU.mult, op1=ALU.subtract,
                        )
                        negx[tag] = nx
                        t_cur[tag] = ident_r32
                        tt_cur[tag] = ident_r32
                    for j in range(EXPM_TERMS, 0, -1):
                        for _, tag in specs:
                            psa = psp.tile([N, N], F32, tag="ps", bufs=4)
                            nc.tensor.matmul(
                                psa[:], negx[tag][:], t_cur[tag][:],
                                start=True, stop=True,
                            )
                            t_new = wp.tile([N, N], F32R, tag=f"T_{tag}",
                                            bufs=2, name=f"T_{tag}")
                            nc.vector.scalar_tensor_tensor(
                                t_new[:], psa[:], 1.0 / j, ident_r32[:],
                                op0=ALU.mult, op1=ALU.add,
                            )
                            t_cur[tag] = t_new
                    for _, tag in specs:
                        pst = psp.tile([N, N], F32R, tag="ps", bufs=4,
                                       name=f"ptt_{tag}")
                        nc.tensor.transpose(pst[:], t_cur[tag][:], ident_r32[:])
                        tt_new = wp.tile([N, N], F32R, tag=f"TT_{tag}",
                                         bufs=2, name=f"TT_{tag}")
                        nc.scalar.copy(tt_new[:], pst[:])
                        tt_cur[tag] = tt_new
                    for _ in range(EXPM_SCAL):
                        for _, tag in specs:
                            psa = psp.tile([N, N], F32, tag="ps", bufs=4)
                            psb = psp.tile([N, N], F32, tag="ps", bufs=4)
                            nc.tensor.matmul(
                                psa[:], tt_cur[tag][:], t_cur[tag][:],
                                start=True, stop=True,
                            )
                            nc.tensor.matmul(
                                psb[:], t_cur[tag][:], tt_cur[tag][:],
                                start=True, stop=True,
                            )
                            t_new = wp.tile([N, N], F32R, tag=f"T_{tag}",
                                            bufs=2, name=f"T_{tag}")
                            tt_new = wp.tile([N, N], F32R, tag=f"TT_{tag}",
                                             bufs=2, name=f"TT_{tag}")
                            nc.vector.tensor_copy(t_new[:], psa[:])
                            nc.scalar.copy(tt_new[:], psb[:])
                            t_cur[tag], tt_cur[tag] = t_new, tt_new
                    return tt_cur

                eres = expm_batch([
                    (zt["ZC_U"], "UCT"), (zt["ZC_V"], "VCT"),
                    (zt["ZB_U"], "UBT"), (zt["ZB_V"], "VBT"),
                    (zt["ZA_U"], "UAT"),
                ])
                uct, vct = eres["UCT"], eres["VCT"]
                ubt, vbt = eres["UBT"], eres["VBT"]
                uat = eres["UAT"]

                def absdiag_col(z_tile, tag):
                    tmp = wp.tile([N, N], F32, tag="us")
                    nc.vector.tensor_mul(tmp[:], z_tile[:], ident[:])
                    col = sp.tile([N, 1], F32, tag=tag, name=f"col_{tag}")
                    nc.vector.tensor_reduce(
                        col[:], tmp[:], AXIS.X, ALU.add,
                        apply_absolute_value=True
                    )
                    return col

                dc_col = absdiag_col(zt["ZC_S"], "dc")
                db_col = absdiag_col(zt["ZB_S"], "db")
                ga_col = absdiag_col(zt["ZA_G"], "ga")

                # top-4: alpha = sqrt(sum_i (b_i c_i)^2)
                bwork = sp.tile([N, 1], F32, tag="bwork")
                cwork = sp.tile([N, 1], F32, tag="cwork")
                nc.vector.tensor_copy(bwork[:], db_col[:])
                nc.vector.tensor_copy(cwork[:], dc_col[:])
                acc = sp.tile([N, 1], F32, tag="acc")
                nc.vector.memset(acc[:], 0.0)
                bmax = sp.tile([N, 1], F32, tag="bmax")
                cmax = sp.tile([N, 1], F32, tag="cmax")
                prod = sp.tile([N, 1], F32, tag="prod")
                gmask = sp.tile([N, 1], F32, tag="gmask")
                tdrop = sp.tile([N, 1], F32, tag="tdrop")
                for i in range(KTOP):
                    nc.gpsimd.partition_all_reduce(
                        bmax[:], bwork[:], N, bass_isa.ReduceOp.max
                    )
                    nc.gpsimd.partition_all_reduce(
                        cmax[:], cwork[:], N, bass_isa.ReduceOp.max
                    )
                    nc.vector.tensor_mul(prod[:], bmax[:], cmax[:])
                    nc.vector.tensor_mul(prod[:], prod[:], prod[:])
                    nc.vector.tensor_add(acc[:], acc[:], prod[:])
                    if i < KTOP - 1:
                        nc.vector.tensor_single_scalar(
                            gmask[:], bwork[:], bmax[:], ALU.is_ge
                        )
                        nc.vector.tensor_mul(tdrop[:], bwork[:], gmask[:])
                        nc.vector.tensor_sub(bwork[:], bwork[:], tdrop[:])
                        nc.vector.tensor_single_scalar(
                            gmask[:], cwork[:], cmax[:], ALU.is_ge
                        )
                        nc.vector.tensor_mul(tdrop[:], cwork[:], gmask[:])
                        nc.vector.tensor_sub(cwork[:], cwork[:], tdrop[:])
                alpha = sp.tile([N, 1], F32, tag="alpha")
                nc.scalar.activation(alpha[:], acc[:], ACTF.Sqrt)

                sa05 = sp.tile([N, 1], F32, tag="sa05")
                nc.vector.tensor_scalar(
                    sa05[:], ga_col[:], alpha[:], -0.5,
                    op0=ALU.add, op1=ALU.mult
                )
                sb01 = sp.tile([N, 1], F32, tag="sb01")
                nc.vector.tensor_scalar_mul(sb01[:], db_col[:], STEP)

                # C^T = VC @ (SC @ UC^T)
                p1 = wp.tile([N, N], F32R, tag="us_r", name="p1")
                nc.vector.tensor_scalar_mul(p1[:], uct[:], dc_col[:])
                psa = psp.tile([N, N], F32, tag="ps", bufs=4)
                nc.tensor.matmul(psa[:], vct[:], p1[:], start=True, stop=True)
                nc.vector.tensor_copy(CTf32[:], psa[:])
                nc.scalar.copy(CTb[:], psa[:])

                # untransposed 0.01 B = UB @ (0.01 SB @ VB^T)
                p2b = wp.tile([N, N], F32R, tag="us_r", name="p2b")
                nc.vector.tensor_scalar_mul(p2b[:], vbt[:], sb01[:])
                psb2 = psp.tile([N, N], F32, tag="ps", bufs=4)
                nc.tensor.matmul(psb2[:], ubt[:], p2b[:], start=True, stop=True)
                bp_un = ep.tile([N, N], F32, tag="Bpun")
                nc.vector.tensor_copy(bp_un[:], psb2[:])

                # M = UA @ (sa05 * UA^T) = 0.5*UA SA UA^T (symmetric)
                p3 = wp.tile([N, N], F32R, tag="us_r", name="p3")
                nc.vector.tensor_scalar_mul(p3[:], uat[:], sa05[:])
                psm = psp.tile([N, N], F32, tag="ps", bufs=4)
                nc.tensor.matmul(psm[:], uat[:], p3[:], start=True, stop=True)
                # YA = Uy - Uy^T; q2 = -0.005*YA
                uy = wp.tile([N, N], F32, tag="us")
                nc.vector.tensor_mul(uy[:], zt["ZA_Y"][:], masku[:])
                pst2 = psp.tile([N, N], F32, tag="ps", bufs=4)
                nc.tensor.transpose(pst2[:], uy[:], ident[:])
                nc.vector.tensor_scalar_mul(uy[:], uy[:], 0.5 * STEP)
                q2 = wp.tile([N, N], F32, tag="T")
                nc.vector.scalar_tensor_tensor(
                    q2[:], pst2[:], 0.5 * STEP, uy[:],
                    op0=ALU.mult, op1=ALU.subtract
                )
                # A~^T = I + (0.01 A)^T ; A~ un-transposed = I + 0.01 A
                ATp1 = powp.tile([N, N], F32, tag="ATp1", name="ATp1")
                a01T = wp.tile([N, N], F32, tag="a01T", bufs=1)
                nc.vector.scalar_tensor_tensor(
                    a01T[:], psm[:], STEP, q2[:], op0=ALU.mult, op1=ALU.add
                )
                nc.vector.tensor_add(ATp1[:], a01T[:], ident[:])
                a01_un = wp.tile([N, N], F32, tag="a01un", bufs=1)
                nc.vector.scalar_tensor_tensor(
                    a01_un[:], psm[:], STEP, q2[:],
                    op0=ALU.mult, op1=ALU.subtract
                )
                Aun = ep.tile([N, N], F32, tag="Aun")
                nc.vector.tensor_add(Aun[:], a01_un[:], ident[:])

                # ---- power chain ATp[i] = A~T^i, log-depth ----
                ATp = [None] * (KB + 1)
                ATp[1] = ATp1
                Aunp = {1: Aun}

                def _mk_pow(i, lhs_un, rhs_t, mk_un=False):
                    psq = psp.tile([N, N], F32, tag="ps", bufs=4)
                    nc.tensor.matmul(psq[:], lhs_un[:], rhs_t[:],
                                     start=True, stop=True)
                    t_ = powp.tile([N, N], F32, tag=f"ATp{i}", name=f"ATp{i}")
                    if i % 2 == 0:
                        nc.vector.tensor_copy(t_[:], psq[:])
                    else:
                        nc.scalar.copy(t_[:], psq[:])
                    ATp[i] = t_
                    if i <= KB - 1:
                        nc.scalar.copy(ATpb[:, ds((i - 1) * N, N)], psq[:])

                for lvl in (1, 2, 4, 8, 16):
                    # un-transposed power 2*lvl = (ATp[lvl])^T @ Aun[lvl]
                    for i in range(lvl + 1, 2 * lvl + 1):
                        _mk_pow(i, Aunp[lvl], ATp[i - lvl])
                    if 2 * lvl < KB:
                        psu = psp.tile([N, N], F32, tag="ps", bufs=4)
                        nc.tensor.matmul(psu[:], ATp[lvl][:], Aunp[lvl][:],
                                         start=True, stop=True)
                        u_ = powp.tile([N, N], F32, tag=f"Aun{2 * lvl}",
                                       name=f"Aun{2 * lvl}")
                        nc.vector.tensor_copy(u_[:], psu[:])
                        Aunp[2 * lvl] = u_
                nc.gpsimd.tensor_copy(ATpb[:, ds(0, N)], ATp1[:])

                # D^T = A~T^K - I
                nc.vector.tensor_sub(DT16[:], ATp[KB][:], ident[:])
                nc.vector.tensor_sub(DT15[:], ATp[KB - 1][:], ident[:])

                # ---- G_i^T = (0.01B)^T A~T^i, i=0..15 ----
                for i in range(KB):
                    psg = psp.tile([N, N], F32, tag="ps", bufs=4,
                                   name=f"G{i}")
                    rhs = ident if i == 0 else ATp[i]
                    nc.tensor.matmul(psg[:], bp_un[:], rhs[:],
                                     start=True, stop=True)
                    if i % 2 == 0:
                        nc.scalar.copy(Gb[:, ds(i * N, N)], psg[:])
                    else:
                        nc.vector.tensor_copy(Gb[:, ds(i * N, N)], psg[:])

                # ---- H_s^T = sum_{i<=s} G_i^T (pairing) ----
                nc.gpsimd.tensor_copy(Hb[:, ds(0, N)], Gb[:, ds(0, N)])
                nc.gpsimd.tensor_add(Hb[:, ds(N, N)], Gb[:, ds(0, N)],
                                     Gb[:, ds(N, N)])
                npair = KB // 2 - 1
                gpair = ep.tile([N, npair * N], BF16, tag="gpair")
                for k in range(1, npair):
                    nc.vector.tensor_add(
                        gpair[:, ds(k * N, N)],
                        Gb[:, ds(2 * k * N, N)],
                        Gb[:, ds((2 * k + 1) * N, N)])
                for k in range(1, KB // 2):
                    # serial chain on odd H; even H branches on Pool
                    nc.gpsimd.tensor_add(
                        Hb[:, ds(2 * k * N, N)],
                        Hb[:, ds((2 * k - 1) * N, N)],
                        Gb[:, ds(2 * k * N, N)])
                    if 2 * k + 1 <= KB - 2:
                        nc.vector.tensor_add(
                            Hb[:, ds((2 * k + 1) * N, N)],
                            Hb[:, ds((2 * k - 1) * N, N)],
                            gpair[:, ds(k * N, N)])

                # ---- c columns: p_s = A~^s bx' then prefix sum ----
                pcols = sp.tile([N, KB], F32, tag="pcols")
                for s in range(KB):
                    lhs = ident if s == 0 else ATp[s]
                    pc = psp.tile([N, N], F32, tag="ps", bufs=4,
                                  name=f"pc{s}")
                    nc.tensor.matmul(pc[:, 0:BSH], lhs[:],
                                     bxp_r[:], start=True, stop=True)
                    eng = nc.vector if s % 2 == 0 else nc.scalar
                    if s % 2 == 0:
                        nc.vector.tensor_copy(pcols[:, s:s + 1], pc[:, 0:1])
                    else:
                        nc.scalar.copy(pcols[:, s:s + 1], pc[:, 0:1])
                ccols = sp.tile([N, KB], F32, tag="ccols")
                onesc = sp.tile([N, KB], F32, tag="onesc")
                nc.vector.memset(onesc[:], 1.0)
                nc.vector.tensor_tensor_scan(
                    ccols[:], onesc[:], pcols[:], 0.0,
                    op0=ALU.mult, op1=ALU.add,
                )
                # transpose -> Cmat16 [16, 128] bf16
                cpst = psp.tile([KB, N], F32, tag="cs2", bufs=1, name="cpst")
                nc.tensor.transpose(cpst[:], ccols[:], ident[:])
                nc.scalar.copy(Cmat16[:], cpst[:])
                c15ps = psp.tile([1, N], F32, tag="cs3", bufs=2, name="c15ps")
                nc.tensor.transpose(c15ps[:], ccols[:, KB - 1:KB], ident[:])
                nc.scalar.copy(c15row[:], c15ps[:])
                c14ps = psp.tile([1, N], F32, tag="cs3", bufs=2, name="c14ps")
                nc.tensor.transpose(c14ps[:], ccols[:, KB - 2:KB - 1], ident[:])
                nc.scalar.copy(c14row[:], c14ps[:])

            # ------- block loop -------
            with (
                tc.tile_pool(name="xsb", bufs=2) as xsbp,
                tc.tile_pool(name="stg", bufs=2) as stgp,
                tc.tile_pool(name="thb", bufs=2) as thp,
                tc.tile_pool(name="base", bufs=2) as basep,
                tc.tile_pool(name="xps", bufs=1, space="PSUM") as xpsp,
                tc.tile_pool(name="psy", bufs=2, space="PSUM") as psyp,
                tc.tile_pool(name="dps", bufs=1, space="PSUM") as dpsp,
                tc.tile_pool(name="trp", bufs=1, space="PSUM") as trpp,
            ):
                # init: th_init = tanh(C x0 + by), base = x0
                psy0 = psyp.tile([N, BSH], F32, tag="psyq", name="psy0")
                nc.tensor.matmul(psy0[:], CTf32[:], x0_c[:],
                                 start=True, stop=True)
                th_init = thp.tile([N, KB * BSH], BF16, tag="th",
                                   name="th_init")
                nc.scalar.activation(
                    th_init[:, ds((KB - 1) * BSH, BSH)], psy0[:],
                    ACTF.Tanh, bias=by_c[:], scale=1.0
                )
                base_cur = basep.tile([N, BSH], F32, tag="base",
                                      name="base0")
                nc.vector.tensor_copy(base_cur[:], x0_c[:])
                xsb_cur = xsbp.tile([N, KB * BSH], BF16, tag="xsb",
                                    name="xsb0")
                nc.scalar.copy(xsb_cur[:, ds(0, BSH)], x0_c[:])
                th_prev = th_init

                for j in range(NBLK):
                    kb = KB if j < NBLK - 1 else KLAST
                    ncols = (kb - 1) * BSH
                    th1 = th_prev[:, ds((KB - 1) * BSH, BSH)]
                    base_b = xsb_cur[:, ds(0, BSH)]
                    xq = xpsp.tile([N, (KB - 1) * BSH], F32, tag="xq")
                    # consts first (opens accumulation, no deps)
                    for off in range(0, ncols, 512):
                        cwid = min(512, ncols - off)
                        nc.tensor.matmul(
                            xq[:, ds(off, cwid)], Cmat16[:],
                            ind16[:, ds(off, cwid)],
                            start=True, stop=False, skip_group_check=True)
                    for s in range(kb - 1):
                        nc.tensor.matmul(
                            xq[:, ds(s * BSH, BSH)],
                            ATpb[:, ds(s * N, N)], base_b,
                            start=False, stop=False, skip_group_check=True)
                        nc.tensor.matmul(
                            xq[:, ds(s * BSH, BSH)],
                            Hb[:, ds(s * N, N)], th1,
                            start=False, stop=True, skip_group_check=True)
                    # quarters: cast -> psy -> tanh
                    th_cur = thp.tile([N, KB * BSH], BF16, tag="th")
                    nq = KB // 4
                    qbounds = [4 * i for i in range(nq)] + [kb]
                    for q in range(nq):
                        s0, s1 = qbounds[q], qbounds[q + 1]
                        w = (s1 - s0) * BSH
                        cw = min(s1, kb - 1) - s0
                        if cw > 0:
                            nc.vector.tensor_copy(
                                xsb_cur[:, ds((1 + s0) * BSH, cw * BSH)],
                                xq[:, ds(s0 * BSH, cw * BSH)],
                            )
                        psyq = psyp.tile([N, KB // 4 * BSH], F32, tag="psyq")
                        nc.tensor.matmul(
                            psyq[:, ds(0, w)], CTb[:],
                            xsb_cur[:, ds(s0 * BSH, w)],
                            start=True, stop=True)
                        nc.scalar.activation(
                            th_cur[:, ds(s0 * BSH, w)], psyq[:, ds(0, w)],
                            ACTF.Tanh, bias=by_c[:], scale=1.0
                        )
                    # refine: Delta = D x_base + sum G_i th_{kb-1-i} + c_{kb-1}
                    dq = dpsp.tile([N, BSH], F32, tag="dq")
                    nc.tensor.matmul(dq[:], DT16[:] if kb == KB else DT15[:],
                                     base_cur[:], start=True, stop=False,
                                     skip_group_check=True)
                    nc.tensor.matmul(dq[:], (c15row if kb == KB else c14row)[:],
                                     ones64[:], start=False, stop=False,
                                     skip_group_check=True)
                    for s in range(kb - 1, -1, -1):
                        i = kb - 1 - s
                        nc.tensor.matmul(
                            dq[:], Gb[:, ds(i * N, N)],
                            th_cur[:, ds(s * BSH, BSH)],
                            start=False, stop=(s == 0),
                            skip_group_check=True)
                    base_new = basep.tile([N, BSH], F32, tag="base")
                    nc.vector.tensor_add(base_new[:], dq[:], base_cur[:])
                    if j < NBLK - 1:
                        xsb_new = xsbp.tile([N, KB * BSH], BF16, tag="xsb")
                        nc.vector.tensor_add(
                            xsb_new[:, ds(0, BSH)], dq[:], base_cur[:])
                    else:
                        # last block: x_511 goes into this block's col 15
                        xsb_new = None
                        nc.vector.tensor_add(
                            xsb_cur[:, ds((KB - 1) * BSH, BSH)],
                            dq[:], base_cur[:])
                    # output transposes: pairs (2i, 2i+1) of xsb_cur cols
                    t0 = 1 + KB * j
                    for h in range(2):
                        trp = trpp.tile([N, 8 * N], BF16, tag="trp")
                        for i in range(8):
                            nc.tensor.transpose(
                                trp[:, ds(i * N, N)],
                                xsb_cur[:, ds((16 * h + 2 * i) * BSH,
                                              2 * BSH)],
                                identb[:],
                            )
                        stg = stgp.tile([N, 8 * N], BF16, tag="stg")
                        nc.vector.tensor_copy(stg[:], trp[:])
                        nc.sync.dma_start(
                            out=out_e[:, ds((t0 - 1) // 2 + 8 * h, 8),
                                      :].rearrange("b i n -> b (i n)"),
                            in_=stg[0:BSH, :],
                        )
                        nc.sync.dma_start(
                            out=out_o[:, ds((t0 - 1) // 2 + 8 * h, 8),
                                      :].rearrange("b i n -> b (i n)"),
                            in_=stg[BSH:2 * BSH, :],
                        )
                    base_cur = base_new
                    th_prev = th_cur
                    xsb_cur = xsb_new

    nc.compile()
    return nc


_CACHED = {}


def _get_program(*_args, **_kw):
    if "p" not in _CACHED:
        _CACHED["p"] = build_program()
    return _CACHED["p"]


def make_in_maps(inputs):
    X0 = np.ascontiguousarray(np.asarray(inputs["X0"], dtype=np.float32))
    base = {
        name: np.ascontiguousarray(np.asarray(inputs[name], dtype=np.float32))
        for name in PARAM_NAMES
    }
    base["bx"] = np.ascontiguousarray(
        np.asarray(inputs["bx"], dtype=np.float32).reshape(N, 1)
    )
    base["by"] = np.ascontiguousarray(
        np.asarray(inputs["by"], dtype=np.float32).reshape(N, 1)
    )
    in_maps = []
    for c in range(NCORES):
        m = dict(base)
        m["x0"] = np.ascontiguousarray(X0[c * BSH:(c + 1) * BSH].T)
        in_maps.append(m)
    return in_maps


def run_spmd(inputs, *_args, trace=False, tmpdir=None, **_kw):
    nc = _get_program()
    in_maps = make_in_maps(inputs)
    res = run_bass_kernel_spmd(
        nc, in_maps, list(range(NCORES)), trace=trace, tmpdir=tmpdir
    )
    outs = []
    for c in range(NCORES):
        ev = np.asarray(res.results[c]["out_e"]).astype(np.float32)
        od = np.asarray(res.results[c]["out_o"]).astype(np.float32)
        full = np.empty((BSH, TMAX, N), dtype=np.float32)
        full[:, 0::2, :] = ev
        full[:, 1::2, :] = od
        outs.append(full)
    return np.concatenate(outs, axis=0), res


def kernel(**inputs):
    full, _ = run_spmd(inputs)
    return full


# revision 7
# speedup vs baseline: 1.2548x; 1.0354x over previous
"""LurieNet-k Trainium2 kernel, block-Picard formulation.

Per-step instruction overheads (ACT tanh ~320ns, DVE ~260-320ns) wall the
baseline per-step recurrence at ~770ns/step.  This kernel processes K=16
steps per block:
  x~_s   = A~^{s+1} x_base + H_s th1 + c_s          (guess th == th1)
  y_s    = C x~_{s-1} + by ;  th_s = tanh(y_s)      (wide quarter tanh)
  Delta  = (A~^K - I) x_base + sum_i G_i th_{K-1-i} + c_{K-1}
  x_base' = x_base + Delta                          (exact fp32 carry)
with A~ = I + 0.01A, G_i = A~^i (0.01B), H_s = sum_{i<=s} G_i, all
precomputed on device.  The constant-th guess contributes O(1e-3) output
error (validated vs reference: rel 2.5e-3); the identity part of the
carry only ever flows through fp32 (D = A~^K - I is applied in f32r and
added to x_base in fp32), so errors do not compound across blocks.

The trajectory is emitted as bf16 via PE pair-transposes into PSUM and
DMA'd straight from PSUM into two DRAM tensors (even/odd timesteps);
the host interleaves and upcasts.
"""

import sys

for _p in ("/opt/trn_rl_repo",):
    if _p not in sys.path:
        sys.path.insert(0, _p)

import numpy as np

import concourse.bass as bass
import concourse.mybir as mybir
import concourse.tile as tile
from concourse import bacc
from concourse import bass_isa
from concourse.bass import ds
from concourse.bass_utils import run_bass_kernel_spmd
from concourse.masks import make_identity, make_upper_triangular

F32 = mybir.dt.float32
F32R = mybir.dt.float32r
BF16 = mybir.dt.bfloat16
ALU = mybir.AluOpType
ACTF = mybir.ActivationFunctionType
AXIS = mybir.AxisListType

N = 128          # state dim
TMAX = 512       # time steps (including t=0)
BS = 512         # global batch
NCORES = 8
BSH = BS // NCORES   # 64 batch columns per core
STEP = 0.01
KTOP = 4
KB = 32          # block length (time steps per block)
NBLK = 16        # 15 full blocks + last block of 31 steps
KLAST = 31

EXPM_SCAL = 3
EXPM_TERMS = 4

PARAM_NAMES = [
    "ZA_Y", "ZA_U", "ZA_G", "ZB_U", "ZB_V", "ZB_S", "ZC_U", "ZC_V", "ZC_S",
]


def build_program():
    nc = bacc.Bacc(
        "TRN2",
        target_bir_lowering=False,
        debug=False,
        enable_asserts=False,
        num_devices=NCORES,
    )

    x0 = nc.dram_tensor("x0", [N, BSH], F32, kind="ExternalInput")
    zs = {
        name: nc.dram_tensor(name, [N, N], F32, kind="ExternalInput")
        for name in PARAM_NAMES
    }
    bx_d = nc.dram_tensor("bx", [N, 1], F32, kind="ExternalInput")
    by_d = nc.dram_tensor("by", [N, 1], F32, kind="ExternalInput")
    # even timesteps t=0,2,..,510 and odd t=1,3,..,511, bf16
    out_e = nc.dram_tensor("out_e", [BSH, TMAX // 2, N], BF16,
                           kind="ExternalOutput")
    out_o = nc.dram_tensor("out_o", [BSH, TMAX // 2, N], BF16,
                           kind="ExternalOutput")

    with tile.TileContext(nc) as tc:
        with tc.tile_pool(name="const", bufs=1) as constp:
            ident = constp.tile([N, N], F32, tag="ident")
            make_identity(nc, ident[:])
            masku = constp.tile([N, N], F32, tag="masku")
            make_upper_triangular(nc, masku[:], val=1.0, diag=False)
            ident_r32 = constp.tile([N, N], F32R, tag="ident_r32")
            nc.vector.tensor_copy(ident_r32[:], ident[:])
            identb = constp.tile([N, N], BF16, tag="identb")
            nc.vector.tensor_copy(identb[:], ident[:])

            by_c = constp.tile([N, 1], F32, tag="by")
            nc.scalar.dma_start(out=by_c[:], in_=by_d[:])
            bx_c = constp.tile([N, 1], F32, tag="bxraw")
            nc.gpsimd.dma_start(out=bx_c[:], in_=bx_d[:])
            bxp_c = constp.tile([N, 1], F32, tag="bxp")
            nc.vector.tensor_scalar_mul(bxp_c[:], bx_c[:], STEP)
            bxp_r = constp.tile([N, BSH], F32, tag="bxpr")
            zer64 = constp.tile([N, BSH], F32, tag="zer64")
            nc.vector.memset(zer64[:], 0.0)
            nc.vector.tensor_scalar(
                bxp_r[:], zer64[:], bx_c[:], STEP,
                op0=ALU.add, op1=ALU.mult)
            x0_c = constp.tile([N, BSH], F32, tag="x0c")
            nc.scalar.dma_start(out=x0_c[:], in_=x0[:])

            # wide precomputed-weight tiles (transposed forms for lhsT use)
            ATpb = constp.tile([N, (KB - 1) * N], BF16, tag="ATpb")
            Hb = constp.tile([N, (KB - 1) * N], BF16, tag="Hb")
            Gb = constp.tile([N, KB * N], BF16, tag="Gb")
            DT16 = constp.tile([N, N], F32, tag="DT16")
            DT15 = constp.tile([N, N], F32, tag="DT15")
            CTb = constp.tile([N, N], BF16, tag="CTb")
            CTf32 = constp.tile([N, N], F32, tag="CTf32")
            Cmat16 = constp.tile([KB, N], BF16, tag="Cmat16")
            c15row = constp.tile([1, N], BF16, tag="c15row")  # c_{KB-1}
            c14row = constp.tile([1, N], BF16, tag="c14row")
            ind16 = constp.tile([KB, (KB - 1) * BSH], BF16, tag="ind16")
            ones64 = constp.tile([1, BSH], BF16, tag="ones64")
            nc.vector.memset(ones64[:], 1.0)
            nc.vector.memset(ind16[:], 0.0)
            for s in range(KB - 1):
                nc.gpsimd.dma_start(out=ind16[s:s + 1, ds(s * BSH, BSH)],
                                    in_=ones64[:])

            # ------- setup: expm's + weight assembly -------
            with (
                tc.tile_pool(name="zbuf", bufs=1) as zp,
                tc.tile_pool(name="work", bufs=2) as wp,
                tc.tile_pool(name="eres", bufs=1) as ep,
                tc.tile_pool(name="small", bufs=1) as sp,
                tc.tile_pool(name="pow", bufs=1) as powp,
                tc.tile_pool(name="pss", bufs=4, space="PSUM") as psp,
            ):
                zt = {}
                _order = ["ZC_U", "ZC_V", "ZB_U", "ZB_V", "ZA_U",
                          "ZC_S", "ZB_S", "ZA_G", "ZA_Y"]
                _qs = {"ZC_U": nc.sync, "ZC_V": nc.sync, "ZB_U": nc.sync,
                       "ZB_V": nc.scalar, "ZA_U": nc.scalar,
                       "ZC_S": nc.sync, "ZB_S": nc.scalar,
                       "ZA_G": nc.gpsimd, "ZA_Y": nc.gpsimd}
                for name in _order:
                    zt[name] = zp.tile([N, N], F32, tag=name, name=f"z_{name}")
                    _qs[name].dma_start(out=zt[name][:], in_=zs[name][:])

                def expm_batch(specs):
                    """Interleaved expm(skew(Z))^T for all matrices at once."""
                    scal = 1.0 / (2.0 ** EXPM_SCAL)
                    negx = {}
                    t_cur = {}
                    tt_cur = {}
                    for z_tile, tag in specs:
                        us = wp.tile([N, N], F32R, tag="us_r", name=f"us_{tag}")
                        nc.vector.scalar_tensor_tensor(
                            us[:], z_tile[:], scal, masku[:],
                            op0=ALU.mult, op1=ALU.mult,
                        )
                        pst = psp.tile([N, N], F32R, tag="ps", bufs=4,
                                       name=f"pst_{tag}")
                        nc.tensor.transpose(pst[:], us[:], ident_r32[:])
                        nx = wp.tile([N, N], F32R, tag=f"negx_{tag}", bufs=1,
                                     name=f"negx_{tag}")
                        nc.vector.scalar_tensor_tensor(
                            nx[:], pst[:], 1.0, us[:],
                            op0=ALU.mult, op1=ALU.subtract,
                        )
                        negx[tag] = nx
                        t_cur[tag] = ident_r32
                        tt_cur[tag] = ident_r32
                    for j in range(EXPM_TERMS, 0, -1):
                        for _, tag in specs:
                            psa = psp.tile([N, N], F32, tag="ps", bufs=4)
                            nc.tensor.matmul(
                                psa[:], negx[tag][:], t_cur[tag][:],
                                start=True, stop=True,
                            )
                            t_new = wp.tile([N, N], F32R, tag=f"T_{tag}",
                                            bufs=2, name=f"T_{tag}")
                            nc.vector.scalar_tensor_tensor(
                                t_new[:], psa[:], 1.0 / j, ident_r32[:],
                                op0=ALU.mult, op1=ALU.add,
                            )
                            t_cur[tag] = t_new
                    for _, tag in specs:
                        pst = psp.tile([N, N], F32R, tag="ps", bufs=4,
                                       name=f"ptt_{tag}")
                        nc.tensor.transpose(pst[:], t_cur[tag][:], ident_r32[:])
                        tt_new = wp.tile([N, N], F32R, tag=f"TT_{tag}",
                                         bufs=2, name=f"TT_{tag}")
                        nc.scalar.copy(tt_new[:], pst[:])
                        tt_cur[tag] = tt_new
                    for _ in range(EXPM_SCAL):
                        for _, tag in specs:
                            psa = psp.tile([N, N], F32, tag="ps", bufs=4)
                            psb = psp.tile([N, N], F32, tag="ps", bufs=4)
                            nc.tensor.matmul(
                                psa[:], tt_cur[tag][:], t_cur[tag][:],
                                start=True, stop=True,
                            )
                            nc.tensor.matmul(
                                psb[:], t_cur[tag][:], tt_cur[tag][:],
                                start=True, stop=True,
                            )
                            t_new = wp.tile([N, N], F32R, tag=f"T_{tag}",
                                            bufs=2, name=f"T_{tag}")
                            tt_new = wp.tile([N, N], F32R, tag=f"TT_{tag}",
                                             bufs=2, name=f"TT_{tag}")
                            nc.vector.tensor_copy(t_new[:], psa[:])
                            nc.scalar.copy(tt_new[:], psb[:])
                            t_cur[tag], tt_cur[tag] = t_new, tt_new
                    return tt_cur

                eres = expm_batch([
                    (zt["ZC_U"], "UCT"), (zt["ZC_V"], "VCT"),
                    (zt["ZB_U"], "UBT"), (zt["ZB_V"], "VBT"),
                    (zt["ZA_U"], "UAT"),
                ])
                uct, vct = eres["UCT"], eres["VCT"]
                ubt, vbt = eres["UBT"], eres["VBT"]
                uat = eres["UAT"]

                def absdiag_col(z_tile, tag):
                    tmp = wp.tile([N, N], F32, tag="us")
                    nc.vector.tensor_mul(tmp[:], z_tile[:], ident[:])
                    col = sp.tile([N, 1], F32, tag=tag, name=f"col_{tag}")
                    nc.vector.tensor_reduce(
                        col[:], tmp[:], AXIS.X, ALU.add,
                        apply_absolute_value=True
                    )
                    return col

                dc_col = absdiag_col(zt["ZC_S"], "dc")
                db_col = absdiag_col(zt["ZB_S"], "db")
                ga_col = absdiag_col(zt["ZA_G"], "ga")

                # top-4: alpha = sqrt(sum_i (b_i c_i)^2)
                bwork = sp.tile([N, 1], F32, tag="bwork")
                cwork = sp.tile([N, 1], F32, tag="cwork")
                nc.vector.tensor_copy(bwork[:], db_col[:])
                nc.vector.tensor_copy(cwork[:], dc_col[:])
                acc = sp.tile([N, 1], F32, tag="acc")
                nc.vector.memset(acc[:], 0.0)
                bmax = sp.tile([N, 1], F32, tag="bmax")
                cmax = sp.tile([N, 1], F32, tag="cmax")
                prod = sp.tile([N, 1], F32, tag="prod")
                gmask = sp.tile([N, 1], F32, tag="gmask")
                tdrop = sp.tile([N, 1], F32, tag="tdrop")
                for i in range(KTOP):
                    nc.gpsimd.partition_all_reduce(
                        bmax[:], bwork[:], N, bass_isa.ReduceOp.max
                    )
                    nc.gpsimd.partition_all_reduce(
                        cmax[:], cwork[:], N, bass_isa.ReduceOp.max
                    )
                    nc.vector.tensor_mul(prod[:], bmax[:], cmax[:])
                    nc.vector.tensor_mul(prod[:], prod[:], prod[:])
                    nc.vector.tensor_add(acc[:], acc[:], prod[:])
                    if i < KTOP - 1:
                        nc.vector.tensor_single_scalar(
                            gmask[:], bwork[:], bmax[:], ALU.is_ge
                        )
                        nc.vector.tensor_mul(tdrop[:], bwork[:], gmask[:])
                        nc.vector.tensor_sub(bwork[:], bwork[:], tdrop[:])
                        nc.vector.tensor_single_scalar(
                            gmask[:], cwork[:], cmax[:], ALU.is_ge
                        )
                        nc.vector.tensor_mul(tdrop[:], cwork[:], gmask[:])
                        nc.vector.tensor_sub(cwork[:], cwork[:], tdrop[:])
                alpha = sp.tile([N, 1], F32, tag="alpha")
                nc.scalar.activation(alpha[:], acc[:], ACTF.Sqrt)

                sa05 = sp.tile([N, 1], F32, tag="sa05")
                nc.vector.tensor_scalar(
                    sa05[:], ga_col[:], alpha[:], -0.5,
                    op0=ALU.add, op1=ALU.mult
                )
                sb01 = sp.tile([N, 1], F32, tag="sb01")
                nc.vector.tensor_scalar_mul(sb01[:], db_col[:], STEP)

                # C^T = VC @ (SC @ UC^T)
                p1 = wp.tile([N, N], F32R, tag="us_r", name="p1")
                nc.vector.tensor_scalar_mul(p1[:], uct[:], dc_col[:])
                psa = psp.tile([N, N], F32, tag="ps", bufs=4)
                nc.tensor.matmul(psa[:], vct[:], p1[:], start=True, stop=True)
                nc.vector.tensor_copy(CTf32[:], psa[:])
                nc.scalar.copy(CTb[:], psa[:])

                # untransposed 0.01 B = UB @ (0.01 SB @ VB^T)
                p2b = wp.tile([N, N], F32R, tag="us_r", name="p2b")
                nc.vector.tensor_scalar_mul(p2b[:], vbt[:], sb01[:])
                psb2 = psp.tile([N, N], F32, tag="ps", bufs=4)
                nc.tensor.matmul(psb2[:], ubt[:], p2b[:], start=True, stop=True)
                bp_un = ep.tile([N, N], F32, tag="Bpun")
                nc.vector.tensor_copy(bp_un[:], psb2[:])

                # M = UA @ (sa05 * UA^T) = 0.5*UA SA UA^T (symmetric)
                p3 = wp.tile([N, N], F32R, tag="us_r", name="p3")
                nc.vector.tensor_scalar_mul(p3[:], uat[:], sa05[:])
                psm = psp.tile([N, N], F32, tag="ps", bufs=4)
                nc.tensor.matmul(psm[:], uat[:], p3[:], start=True, stop=True)
                # YA = Uy - Uy^T; q2 = -0.005*YA
                uy = wp.tile([N, N], F32, tag="us")
                nc.vector.tensor_mul(uy[:], zt["ZA_Y"][:], masku[:])
                pst2 = psp.tile([N, N], F32, tag="ps", bufs=4)
                nc.tensor.transpose(pst2[:], uy[:], ident[:])
                nc.vector.tensor_scalar_mul(uy[:], uy[:], 0.5 * STEP)
                q2 = wp.tile([N, N], F32, tag="T")
                nc.vector.scalar_tensor_tensor(
                    q2[:], pst2[:], 0.5 * STEP, uy[:],
                    op0=ALU.mult, op1=ALU.subtract
                )
                # A~^T = I + (0.01 A)^T ; A~ un-transposed = I + 0.01 A
                ATp1 = powp.tile([N, N], F32, tag="ATp1", name="ATp1")
                a01T = wp.tile([N, N], F32, tag="a01T", bufs=1)
                nc.vector.scalar_tensor_tensor(
                    a01T[:], psm[:], STEP, q2[:], op0=ALU.mult, op1=ALU.add
                )
                nc.vector.tensor_add(ATp1[:], a01T[:], ident[:])
                a01_un = wp.tile([N, N], F32, tag="a01un", bufs=1)
                nc.vector.scalar_tensor_tensor(
                    a01_un[:], psm[:], STEP, q2[:],
                    op0=ALU.mult, op1=ALU.subtract
                )
                Aun = ep.tile([N, N], F32, tag="Aun")
                nc.vector.tensor_add(Aun[:], a01_un[:], ident[:])

                # ---- power chain ATp[i] = A~T^i, log-depth ----
                ATp = [None] * (KB + 1)
                ATp[1] = ATp1
                Aunp = {1: Aun}

                def _mk_pow(i, lhs_un, rhs_t, mk_un=False):
                    psq = psp.tile([N, N], F32, tag="ps", bufs=4)
                    nc.tensor.matmul(psq[:], lhs_un[:], rhs_t[:],
                                     start=True, stop=True)
                    t_ = powp.tile([N, N], F32, tag=f"ATp{i}", name=f"ATp{i}")
                    if i % 2 == 0:
                        nc.vector.tensor_copy(t_[:], psq[:])
                    else:
                        nc.scalar.copy(t_[:], psq[:])
                    ATp[i] = t_
                    if i <= KB - 1:
                        nc.scalar.copy(ATpb[:, ds((i - 1) * N, N)], psq[:])

                for lvl in (1, 2, 4, 8, 16):
                    # un-transposed power 2*lvl = (ATp[lvl])^T @ Aun[lvl]
                    for i in range(lvl + 1, 2 * lvl + 1):
                        _mk_pow(i, Aunp[lvl], ATp[i - lvl])
                    if 2 * lvl < KB:
                        psu = psp.tile([N, N], F32, tag="ps", bufs=4)
                        nc.tensor.matmul(psu[:], ATp[lvl][:], Aunp[lvl][:],
                                         start=True, stop=True)
                        u_ = powp.tile([N, N], F32, tag=f"Aun{2 * lvl}",
                                       name=f"Aun{2 * lvl}")
                        nc.vector.tensor_copy(u_[:], psu[:])
                        Aunp[2 * lvl] = u_
                nc.gpsimd.tensor_copy(ATpb[:, ds(0, N)], ATp1[:])

                # D^T = A~T^K - I
                nc.vector.tensor_sub(DT16[:], ATp[KB][:], ident[:])
                nc.vector.tensor_sub(DT15[:], ATp[KB - 1][:], ident[:])

                # ---- G_i^T = (0.01B)^T A~T^i, i=0..15 ----
                bp_un_b = ep.tile([N, N], BF16, tag="Bpunb")
                nc.vector.tensor_copy(bp_un_b[:], bp_un[:])
                for i in range(KB):
                    psg = psp.tile([N, N], F32, tag="ps", bufs=4,
                                   name=f"G{i}")
                    rhs = identb[:] if i == 0 else ATpb[:, ds((i - 1) * N, N)]
                    nc.tensor.matmul(psg[:], bp_un_b[:], rhs,
                                     start=True, stop=True)
                    if i % 2 == 0:
                        nc.scalar.copy(Gb[:, ds(i * N, N)], psg[:])
                    else:
                        nc.vector.tensor_copy(Gb[:, ds(i * N, N)], psg[:])

                # ---- H_s^T = sum_{i<=s} G_i^T (pairing) ----
                nc.gpsimd.tensor_copy(Hb[:, ds(0, N)], Gb[:, ds(0, N)])
                nc.gpsimd.tensor_add(Hb[:, ds(N, N)], Gb[:, ds(0, N)],
                                     Gb[:, ds(N, N)])
                npair = KB // 2 - 1
                gpair = ep.tile([N, npair * N], BF16, tag="gpair")
                for k in range(1, npair):
                    nc.vector.tensor_add(
                        gpair[:, ds(k * N, N)],
                        Gb[:, ds(2 * k * N, N)],
                        Gb[:, ds((2 * k + 1) * N, N)])
                for k in range(1, KB // 2):
                    # serial chain on odd H; even H branches on Pool
                    nc.gpsimd.tensor_add(
                        Hb[:, ds(2 * k * N, N)],
                        Hb[:, ds((2 * k - 1) * N, N)],
                        Gb[:, ds(2 * k * N, N)])
                    if 2 * k + 1 <= KB - 2:
                        nc.vector.tensor_add(
                            Hb[:, ds((2 * k + 1) * N, N)],
                            Hb[:, ds((2 * k - 1) * N, N)],
                            gpair[:, ds(k * N, N)])

                # ---- c columns: p_s = A~^s bx' then prefix sum ----
                pcols = sp.tile([N, KB], F32, tag="pcols")
                bxp_b = sp.tile([N, BSH], BF16, tag="bxpb")
                nc.vector.tensor_copy(bxp_b[:], bxp_r[:])
                for s in range(KB):
                    lhs = identb[:] if s == 0 else ATpb[:, ds((s - 1) * N, N)]
                    pc = psp.tile([N, N], F32, tag="ps", bufs=4,
                                  name=f"pc{s}")
                    nc.tensor.matmul(pc[:, 0:BSH], lhs,
                                     bxp_b[:], start=True, stop=True)
                    eng = nc.vector if s % 2 == 0 else nc.scalar
                    if s % 2 == 0:
                        nc.vector.tensor_copy(pcols[:, s:s + 1], pc[:, 0:1])
                    else:
                        nc.scalar.copy(pcols[:, s:s + 1], pc[:, 0:1])
                ccols = sp.tile([N, KB], F32, tag="ccols")
                onesc = sp.tile([N, KB], F32, tag="onesc")
                nc.vector.memset(onesc[:], 1.0)
                nc.vector.tensor_tensor_scan(
                    ccols[:], onesc[:], pcols[:], 0.0,
                    op0=ALU.mult, op1=ALU.add,
                )
                # transpose -> Cmat16 [16, 128] bf16
                cpst = psp.tile([KB, N], F32, tag="cs2", bufs=1, name="cpst")
                nc.tensor.transpose(cpst[:], ccols[:], ident[:])
                nc.scalar.copy(Cmat16[:], cpst[:])
                c15ps = psp.tile([1, N], F32, tag="cs3", bufs=2, name="c15ps")
                nc.tensor.transpose(c15ps[:], ccols[:, KB - 1:KB], ident[:])
                nc.scalar.copy(c15row[:], c15ps[:])
                c14ps = psp.tile([1, N], F32, tag="cs3", bufs=2, name="c14ps")
                nc.tensor.transpose(c14ps[:], ccols[:, KB - 2:KB - 1], ident[:])
                nc.scalar.copy(c14row[:], c14ps[:])

            # ------- block loop -------
            with (
                tc.tile_pool(name="xsb", bufs=2) as xsbp,
                tc.tile_pool(name="stg", bufs=2) as stgp,
                tc.tile_pool(name="thb", bufs=2) as thp,
                tc.tile_pool(name="base", bufs=2) as basep,
                tc.tile_pool(name="xps", bufs=1, space="PSUM") as xpsp,
                tc.tile_pool(name="psy", bufs=2, space="PSUM") as psyp,
                tc.tile_pool(name="dps", bufs=1, space="PSUM") as dpsp,
                tc.tile_pool(name="trp", bufs=1, space="PSUM") as trpp,
            ):
                # init: th_init = tanh(C x0 + by), base = x0
                psy0 = psyp.tile([N, BSH], F32, tag="psyq", name="psy0")
                nc.tensor.matmul(psy0[:], CTf32[:], x0_c[:],
                                 start=True, stop=True)
                th_init = thp.tile([N, KB * BSH], BF16, tag="th",
                                   name="th_init")
                nc.scalar.activation(
                    th_init[:, ds((KB - 1) * BSH, BSH)], psy0[:],
                    ACTF.Tanh, bias=by_c[:], scale=1.0
                )
                base_cur = basep.tile([N, BSH], F32, tag="base",
                                      name="base0")
                nc.vector.tensor_copy(base_cur[:], x0_c[:])
                xsb_cur = xsbp.tile([N, KB * BSH], BF16, tag="xsb",
                                    name="xsb0")
                nc.scalar.copy(xsb_cur[:, ds(0, BSH)], x0_c[:])
                th_prev = th_init

                for j in range(NBLK):
                    kb = KB if j < NBLK - 1 else KLAST
                    ncols = (kb - 1) * BSH
                    th1 = th_prev[:, ds((KB - 1) * BSH, BSH)]
                    base_b = xsb_cur[:, ds(0, BSH)]
                    xq = xpsp.tile([N, (KB - 1) * BSH], F32, tag="xq")
                    # consts first (opens accumulation, no deps)
                    for off in range(0, ncols, 512):
                        cwid = min(512, ncols - off)
                        nc.tensor.matmul(
                            xq[:, ds(off, cwid)], Cmat16[:],
                            ind16[:, ds(off, cwid)],
                            start=True, stop=False, skip_group_check=True)
                    for s in range(kb - 1):
                        nc.tensor.matmul(
                            xq[:, ds(s * BSH, BSH)],
                            ATpb[:, ds(s * N, N)], base_b,
                            start=False, stop=False, skip_group_check=True)
                        nc.tensor.matmul(
                            xq[:, ds(s * BSH, BSH)],
                            Hb[:, ds(s * N, N)], th1,
                            start=False, stop=True, skip_group_check=True)
                    # quarters: cast -> psy -> tanh
                    th_cur = thp.tile([N, KB * BSH], BF16, tag="th")
                    nq = KB // 4
                    qbounds = [4 * i for i in range(nq)] + [kb]
                    for q in range(nq):
                        s0, s1 = qbounds[q], qbounds[q + 1]
                        w = (s1 - s0) * BSH
                        cw = min(s1, kb - 1) - s0
                        if cw > 0:
                            nc.vector.tensor_copy(
                                xsb_cur[:, ds((1 + s0) * BSH, cw * BSH)],
                                xq[:, ds(s0 * BSH, cw * BSH)],
                            )
                        psyq = psyp.tile([N, KB // 4 * BSH], F32, tag="psyq")
                        nc.tensor.matmul(
                            psyq[:, ds(0, w)], CTb[:],
                            xsb_cur[:, ds(s0 * BSH, w)],
                            start=True, stop=True)
                        nc.scalar.activation(
                            th_cur[:, ds(s0 * BSH, w)], psyq[:, ds(0, w)],
                            ACTF.Tanh, bias=by_c[:], scale=1.0
                        )
                    # refine: Delta = D x_base + sum G_i th_{kb-1-i} + c_{kb-1}
                    dq = dpsp.tile([N, BSH], F32, tag="dq")
                    nc.tensor.matmul(dq[:], DT16[:] if kb == KB else DT15[:],
                                     base_cur[:], start=True, stop=False,
                                     skip_group_check=True)
                    nc.tensor.matmul(dq[:], (c15row if kb == KB else c14row)[:],
                                     ones64[:], start=False, stop=False,
                                     skip_group_check=True)
                    for s in range(kb - 1, -1, -1):
                        i = kb - 1 - s
                        nc.tensor.matmul(
                            dq[:], Gb[:, ds(i * N, N)],
                            th_cur[:, ds(s * BSH, BSH)],
                            start=False, stop=(s == 0),
                            skip_group_check=True)
                    base_new = basep.tile([N, BSH], F32, tag="base")
                    nc.vector.tensor_add(base_new[:], dq[:], base_cur[:])
                    if j < NBLK - 1:
                        xsb_new = xsbp.tile([N, KB * BSH], BF16, tag="xsb")
                        nc.vector.tensor_add(
                            xsb_new[:, ds(0, BSH)], dq[:], base_cur[:])
                    else:
                        # last block: x_511 goes into this block's col 15
                        xsb_new = None
                        nc.vector.tensor_add(
                            xsb_cur[:, ds((KB - 1) * BSH, BSH)],
                            dq[:], base_cur[:])
                    # output transposes: pairs (2i, 2i+1) of xsb_cur cols
                    t0 = 1 + KB * j
                    for h in range(2):
                        trp = trpp.tile([N, 8 * N], BF16, tag="trp")
                        for i in range(8):
                            nc.tensor.transpose(
                                trp[:, ds(i * N, N)],
                                xsb_cur[:, ds((16 * h + 2 * i) * BSH,
                                              2 * BSH)],
                                identb[:],
                            )
                        stg = stgp.tile([N, 8 * N], BF16, tag="stg")
                        nc.vector.tensor_copy(stg[:], trp[:])
                        nc.sync.dma_start(
                            out=out_e[:, ds((t0 - 1) // 2 + 8 * h, 8),
                                      :].rearrange("b i n -> b (i n)"),
                            in_=stg[0:BSH, :],
                        )
                        nc.sync.dma_start(
                            out=out_o[:, ds((t0 - 1) // 2 + 8 * h, 8),
                                      :].rearrange("b i n -> b (i n)"),
                            in_=stg[BSH:2 * BSH, :],
                        )
                    base_cur = base_new
                    th_prev = th_cur
                    xsb_cur = xsb_new

    nc.compile()
    return nc


_CACHED = {}


def _get_program(*_args, **_kw):
    if "p" not in _CACHED:
        _CACHED["p"] = build_program()
    return _CACHED["p"]


def make_in_maps(inputs):
    X0 = np.ascontiguousarray(np.asarray(inputs["X0"], dtype=np.float32))
    base = {
        name: np.ascontiguousarray(np.asarray(inputs[name], dtype=np.float32))
        for name in PARAM_NAMES
    }
    base["bx"] = np.ascontiguousarray(
        np.asarray(inputs["bx"], dtype=np.float32).reshape(N, 1)
    )
    base["by"] = np.ascontiguousarray(
        np.asarray(inputs["by"], dtype=np.float32).reshape(N, 1)
    )
    in_maps = []
    for c in range(NCORES):
        m = dict(base)
        m["x0"] = np.ascontiguousarray(X0[c * BSH:(c + 1) * BSH].T)
        in_maps.append(m)
    return in_maps


def run_spmd(inputs, *_args, trace=False, tmpdir=None, **_kw):
    nc = _get_program()
    in_maps = make_in_maps(inputs)
    res = run_bass_kernel_spmd(
        nc, in_maps, list(range(NCORES)), trace=trace, tmpdir=tmpdir
    )
    outs = []
    for c in range(NCORES):
        ev = np.asarray(res.results[c]["out_e"]).astype(np.float32)
        od = np.asarray(res.results[c]["out_o"]).astype(np.float32)
        full = np.empty((BSH, TMAX, N), dtype=np.float32)
        full[:, 0::2, :] = ev
        full[:, 1::2, :] = od
        outs.append(full)
    return np.concatenate(outs, axis=0), res


def kernel(**inputs):
    full, _ = run_spmd(inputs)
    return full


# revision 8
# speedup vs baseline: 1.2695x; 1.0117x over previous
"""LurieNet-k Trainium2 kernel, block-Picard formulation.

Per-step instruction overheads (ACT tanh ~320ns, DVE ~260-320ns) wall the
baseline per-step recurrence at ~770ns/step.  This kernel processes K=16
steps per block:
  x~_s   = A~^{s+1} x_base + H_s th1 + c_s          (guess th == th1)
  y_s    = C x~_{s-1} + by ;  th_s = tanh(y_s)      (wide quarter tanh)
  Delta  = (A~^K - I) x_base + sum_i G_i th_{K-1-i} + c_{K-1}
  x_base' = x_base + Delta                          (exact fp32 carry)
with A~ = I + 0.01A, G_i = A~^i (0.01B), H_s = sum_{i<=s} G_i, all
precomputed on device.  The constant-th guess contributes O(1e-3) output
error (validated vs reference: rel 2.5e-3); the identity part of the
carry only ever flows through fp32 (D = A~^K - I is applied in f32r and
added to x_base in fp32), so errors do not compound across blocks.

The trajectory is emitted as bf16 via PE pair-transposes into PSUM and
DMA'd straight from PSUM into two DRAM tensors (even/odd timesteps);
the host interleaves and upcasts.
"""

import sys

for _p in ("/opt/trn_rl_repo",):
    if _p not in sys.path:
        sys.path.insert(0, _p)

import numpy as np

import concourse.bass as bass
import concourse.mybir as mybir
import concourse.tile as tile
from concourse import bacc
from concourse import bass_isa
from concourse.bass import ds
from concourse.bass_utils import run_bass_kernel_spmd
from concourse.masks import make_identity, make_upper_triangular

F32 = mybir.dt.float32
F32R = mybir.dt.float32r
BF16 = mybir.dt.bfloat16
ALU = mybir.AluOpType
ACTF = mybir.ActivationFunctionType
AXIS = mybir.AxisListType

N = 128          # state dim
TMAX = 512       # time steps (including t=0)
BS = 512         # global batch
NCORES = 8
BSH = BS // NCORES   # 64 batch columns per core
STEP = 0.01
KTOP = 4
KB = 32          # block length (time steps per block)
NBLK = 16        # 15 full blocks + last block of 31 steps
KLAST = 31

EXPM_SCAL = 3
EXPM_TERMS = 4

PARAM_NAMES = [
    "ZA_Y", "ZA_U", "ZA_G", "ZB_U", "ZB_V", "ZB_S", "ZC_U", "ZC_V", "ZC_S",
]


def build_program():
    nc = bacc.Bacc(
        "TRN2",
        target_bir_lowering=False,
        debug=False,
        enable_asserts=False,
        num_devices=NCORES,
    )

    x0 = nc.dram_tensor("x0", [N, BSH], F32, kind="ExternalInput")
    zs = {
        name: nc.dram_tensor(name, [N, N], F32, kind="ExternalInput")
        for name in PARAM_NAMES
    }
    bx_d = nc.dram_tensor("bx", [N, 1], F32, kind="ExternalInput")
    by_d = nc.dram_tensor("by", [N, 1], F32, kind="ExternalInput")
    # even timesteps t=0,2,..,510 and odd t=1,3,..,511, bf16
    out_e = nc.dram_tensor("out_e", [BSH, TMAX // 2, N], BF16,
                           kind="ExternalOutput")
    out_o = nc.dram_tensor("out_o", [BSH, TMAX // 2, N], BF16,
                           kind="ExternalOutput")

    with tile.TileContext(nc) as tc:
        with tc.tile_pool(name="const", bufs=1) as constp:
            ident = constp.tile([N, N], F32, tag="ident")
            make_identity(nc, ident[:])
            masku = constp.tile([N, N], F32, tag="masku")
            make_upper_triangular(nc, masku[:], val=1.0, diag=False)
            ident_r32 = constp.tile([N, N], F32R, tag="ident_r32")
            nc.vector.tensor_copy(ident_r32[:], ident[:])
            identb = constp.tile([N, N], BF16, tag="identb")
            nc.vector.tensor_copy(identb[:], ident[:])

            by_c = constp.tile([N, 1], F32, tag="by")
            nc.scalar.dma_start(out=by_c[:], in_=by_d[:])
            bx_c = constp.tile([N, 1], F32, tag="bxraw")
            nc.gpsimd.dma_start(out=bx_c[:], in_=bx_d[:])
            bxp_c = constp.tile([N, 1], F32, tag="bxp")
            nc.vector.tensor_scalar_mul(bxp_c[:], bx_c[:], STEP)
            bxp_r = constp.tile([N, BSH], F32, tag="bxpr")
            zer64 = constp.tile([N, BSH], F32, tag="zer64")
            nc.vector.memset(zer64[:], 0.0)
            nc.vector.tensor_scalar(
                bxp_r[:], zer64[:], bx_c[:], STEP,
                op0=ALU.add, op1=ALU.mult)
            x0_c = constp.tile([N, BSH], F32, tag="x0c")
            nc.scalar.dma_start(out=x0_c[:], in_=x0[:])

            # wide precomputed-weight tiles (transposed forms for lhsT use)
            ATpb = constp.tile([N, (KB - 1) * N], BF16, tag="ATpb")
            Hb = constp.tile([N, (KB - 1) * N], BF16, tag="Hb")
            Gb = constp.tile([N, KB * N], BF16, tag="Gb")
            DT16 = constp.tile([N, N], F32, tag="DT16")
            DT15 = constp.tile([N, N], F32, tag="DT15")
            CTb = constp.tile([N, N], BF16, tag="CTb")
            CTf32 = constp.tile([N, N], F32, tag="CTf32")
            Cmat16 = constp.tile([KB, N], BF16, tag="Cmat16")
            c15row = constp.tile([1, N], BF16, tag="c15row")  # c_{KB-1}
            c14row = constp.tile([1, N], BF16, tag="c14row")
            ind16 = constp.tile([KB, (KB - 1) * BSH], BF16, tag="ind16")
            ones64 = constp.tile([1, BSH], BF16, tag="ones64")
            nc.vector.memset(ones64[:], 1.0)
            nc.vector.memset(ind16[:], 0.0)
            for s in range(KB - 1):
                nc.gpsimd.dma_start(out=ind16[s:s + 1, ds(s * BSH, BSH)],
                                    in_=ones64[:])

            # ------- setup: expm's + weight assembly -------
            with (
                tc.tile_pool(name="zbuf", bufs=1) as zp,
                tc.tile_pool(name="work", bufs=2) as wp,
                tc.tile_pool(name="eres", bufs=1) as ep,
                tc.tile_pool(name="small", bufs=1) as sp,
                tc.tile_pool(name="pow", bufs=1) as powp,
                tc.tile_pool(name="pss", bufs=4, space="PSUM") as psp,
            ):
                zt = {}
                _order = ["ZC_U", "ZC_V", "ZB_U", "ZB_V", "ZA_U",
                          "ZC_S", "ZB_S", "ZA_G", "ZA_Y"]
                _qs = {"ZC_U": nc.sync, "ZC_V": nc.sync, "ZB_U": nc.sync,
                       "ZB_V": nc.scalar, "ZA_U": nc.scalar,
                       "ZC_S": nc.sync, "ZB_S": nc.scalar,
                       "ZA_G": nc.gpsimd, "ZA_Y": nc.gpsimd}
                for name in _order:
                    zt[name] = zp.tile([N, N], F32, tag=name, name=f"z_{name}")
                    _qs[name].dma_start(out=zt[name][:], in_=zs[name][:])

                def expm_batch(specs):
                    """Interleaved expm(skew(Z))^T for all matrices at once."""
                    scal = 1.0 / (2.0 ** EXPM_SCAL)
                    negx = {}
                    t_cur = {}
                    tt_cur = {}
                    for z_tile, tag in specs:
                        us = wp.tile([N, N], F32R, tag="us_r", name=f"us_{tag}")
                        nc.vector.scalar_tensor_tensor(
                            us[:], z_tile[:], scal, masku[:],
                            op0=ALU.mult, op1=ALU.mult,
                        )
                        pst = psp.tile([N, N], F32R, tag="ps", bufs=4,
                                       name=f"pst_{tag}")
                        nc.tensor.transpose(pst[:], us[:], ident_r32[:])
                        nx = wp.tile([N, N], F32R, tag=f"negx_{tag}", bufs=1,
                                     name=f"negx_{tag}")
                        nc.vector.scalar_tensor_tensor(
                            nx[:], pst[:], 1.0, us[:],
                            op0=ALU.mult, op1=ALU.subtract,
                        )
                        negx[tag] = nx
                        t_cur[tag] = ident_r32
                        tt_cur[tag] = ident_r32
                    for j in range(EXPM_TERMS, 0, -1):
                        for _, tag in specs:
                            psa = psp.tile([N, N], F32, tag="ps", bufs=4)
                            nc.tensor.matmul(
                                psa[:], negx[tag][:], t_cur[tag][:],
                                start=True, stop=True,
                            )
                            t_new = wp.tile([N, N], F32R, tag=f"T_{tag}",
                                            bufs=2, name=f"T_{tag}")
                            nc.vector.scalar_tensor_tensor(
                                t_new[:], psa[:], 1.0 / j, ident_r32[:],
                                op0=ALU.mult, op1=ALU.add,
                            )
                            t_cur[tag] = t_new
                    for _, tag in specs:
                        pst = psp.tile([N, N], F32R, tag="ps", bufs=4,
                                       name=f"ptt_{tag}")
                        nc.tensor.transpose(pst[:], t_cur[tag][:], ident_r32[:])
                        tt_new = wp.tile([N, N], F32R, tag=f"TT_{tag}",
                                         bufs=2, name=f"TT_{tag}")
                        nc.scalar.copy(tt_new[:], pst[:])
                        tt_cur[tag] = tt_new
                    for _ in range(EXPM_SCAL):
                        for _, tag in specs:
                            psa = psp.tile([N, N], F32, tag="ps", bufs=4)
                            psb = psp.tile([N, N], F32, tag="ps", bufs=4)
                            nc.tensor.matmul(
                                psa[:], tt_cur[tag][:], t_cur[tag][:],
                                start=True, stop=True,
                            )
                            nc.tensor.matmul(
                                psb[:], t_cur[tag][:], tt_cur[tag][:],
                                start=True, stop=True,
                            )
                            t_new = wp.tile([N, N], F32R, tag=f"T_{tag}",
                                            bufs=2, name=f"T_{tag}")
                            tt_new = wp.tile([N, N], F32R, tag=f"TT_{tag}",
                                             bufs=2, name=f"TT_{tag}")
                            nc.vector.tensor_copy(t_new[:], psa[:])
                            nc.scalar.copy(tt_new[:], psb[:])
                            t_cur[tag], tt_cur[tag] = t_new, tt_new
                    return tt_cur

                eres = expm_batch([
                    (zt["ZC_U"], "UCT"), (zt["ZC_V"], "VCT"),
                    (zt["ZB_U"], "UBT"), (zt["ZB_V"], "VBT"),
                    (zt["ZA_U"], "UAT"),
                ])
                uct, vct = eres["UCT"], eres["VCT"]
                ubt, vbt = eres["UBT"], eres["VBT"]
                uat = eres["UAT"]

                def absdiag_col(z_tile, tag):
                    tmp = wp.tile([N, N], F32, tag="us")
                    nc.vector.tensor_mul(tmp[:], z_tile[:], ident[:])
                    col = sp.tile([N, 1], F32, tag=tag, name=f"col_{tag}")
                    nc.vector.tensor_reduce(
                        col[:], tmp[:], AXIS.X, ALU.add,
                        apply_absolute_value=True
                    )
                    return col

                dc_col = absdiag_col(zt["ZC_S"], "dc")
                db_col = absdiag_col(zt["ZB_S"], "db")
                ga_col = absdiag_col(zt["ZA_G"], "ga")

                # top-4: alpha = sqrt(sum_i (b_i c_i)^2)
                bwork = sp.tile([N, 1], F32, tag="bwork")
                cwork = sp.tile([N, 1], F32, tag="cwork")
                nc.vector.tensor_copy(bwork[:], db_col[:])
                nc.vector.tensor_copy(cwork[:], dc_col[:])
                acc = sp.tile([N, 1], F32, tag="acc")
                nc.vector.memset(acc[:], 0.0)
                bmax = sp.tile([N, 1], F32, tag="bmax")
                cmax = sp.tile([N, 1], F32, tag="cmax")
                prod = sp.tile([N, 1], F32, tag="prod")
                gmask = sp.tile([N, 1], F32, tag="gmask")
                tdrop = sp.tile([N, 1], F32, tag="tdrop")
                for i in range(KTOP):
                    nc.gpsimd.partition_all_reduce(
                        bmax[:], bwork[:], N, bass_isa.ReduceOp.max
                    )
                    nc.gpsimd.partition_all_reduce(
                        cmax[:], cwork[:], N, bass_isa.ReduceOp.max
                    )
                    nc.vector.tensor_mul(prod[:], bmax[:], cmax[:])
                    nc.vector.tensor_mul(prod[:], prod[:], prod[:])
                    nc.vector.tensor_add(acc[:], acc[:], prod[:])
                    if i < KTOP - 1:
                        nc.vector.tensor_single_scalar(
                            gmask[:], bwork[:], bmax[:], ALU.is_ge
                        )
                        nc.vector.tensor_mul(tdrop[:], bwork[:], gmask[:])
                        nc.vector.tensor_sub(bwork[:], bwork[:], tdrop[:])
                        nc.vector.tensor_single_scalar(
                            gmask[:], cwork[:], cmax[:], ALU.is_ge
                        )
                        nc.vector.tensor_mul(tdrop[:], cwork[:], gmask[:])
                        nc.vector.tensor_sub(cwork[:], cwork[:], tdrop[:])
                alpha = sp.tile([N, 1], F32, tag="alpha")
                nc.scalar.activation(alpha[:], acc[:], ACTF.Sqrt)

                sa05 = sp.tile([N, 1], F32, tag="sa05")
                nc.vector.tensor_scalar(
                    sa05[:], ga_col[:], alpha[:], -0.5,
                    op0=ALU.add, op1=ALU.mult
                )
                sb01 = sp.tile([N, 1], F32, tag="sb01")
                nc.vector.tensor_scalar_mul(sb01[:], db_col[:], STEP)

                # C^T = VC @ (SC @ UC^T)
                p1 = wp.tile([N, N], F32R, tag="us_r", name="p1")
                nc.vector.tensor_scalar_mul(p1[:], uct[:], dc_col[:])
                psa = psp.tile([N, N], F32, tag="ps", bufs=4)
                nc.tensor.matmul(psa[:], vct[:], p1[:], start=True, stop=True)
                nc.vector.tensor_copy(CTf32[:], psa[:])
                nc.scalar.copy(CTb[:], psa[:])

                # untransposed 0.01 B = UB @ (0.01 SB @ VB^T)
                p2b = wp.tile([N, N], F32R, tag="us_r", name="p2b")
                nc.vector.tensor_scalar_mul(p2b[:], vbt[:], sb01[:])
                psb2 = psp.tile([N, N], F32, tag="ps", bufs=4)
                nc.tensor.matmul(psb2[:], ubt[:], p2b[:], start=True, stop=True)
                bp_un = ep.tile([N, N], F32, tag="Bpun")
                nc.vector.tensor_copy(bp_un[:], psb2[:])

                # M = UA @ (sa05 * UA^T) = 0.5*UA SA UA^T (symmetric)
                p3 = wp.tile([N, N], F32R, tag="us_r", name="p3")
                nc.vector.tensor_scalar_mul(p3[:], uat[:], sa05[:])
                psm = psp.tile([N, N], F32, tag="ps", bufs=4)
                nc.tensor.matmul(psm[:], uat[:], p3[:], start=True, stop=True)
                # YA = Uy - Uy^T; q2 = -0.005*YA
                uy = wp.tile([N, N], F32, tag="us")
                nc.vector.tensor_mul(uy[:], zt["ZA_Y"][:], masku[:])
                pst2 = psp.tile([N, N], F32, tag="ps", bufs=4)
                nc.tensor.transpose(pst2[:], uy[:], ident[:])
                nc.vector.tensor_scalar_mul(uy[:], uy[:], 0.5 * STEP)
                q2 = wp.tile([N, N], F32, tag="T")
                nc.vector.scalar_tensor_tensor(
                    q2[:], pst2[:], 0.5 * STEP, uy[:],
                    op0=ALU.mult, op1=ALU.subtract
                )
                # A~^T = I + (0.01 A)^T ; A~ un-transposed = I + 0.01 A
                ATp1 = powp.tile([N, N], F32, tag="ATp1", name="ATp1")
                a01T = wp.tile([N, N], F32, tag="a01T", bufs=1)
                nc.vector.scalar_tensor_tensor(
                    a01T[:], psm[:], STEP, q2[:], op0=ALU.mult, op1=ALU.add
                )
                nc.vector.tensor_add(ATp1[:], a01T[:], ident[:])
                a01_un = wp.tile([N, N], F32, tag="a01un", bufs=1)
                nc.vector.scalar_tensor_tensor(
                    a01_un[:], psm[:], STEP, q2[:],
                    op0=ALU.mult, op1=ALU.subtract
                )
                Aun = ep.tile([N, N], F32, tag="Aun")
                nc.vector.tensor_add(Aun[:], a01_un[:], ident[:])

                # ---- power chain: fp32 squarings (feed D), bf16 branches
                ATp = [None] * (KB + 1)
                ATp[1] = ATp1
                Aunp = {1: Aun}
                Aunb = {}
                Aunb1 = powp.tile([N, N], BF16, tag="Aunb1", name="Aunb1")
                nc.vector.tensor_copy(Aunb1[:], Aun[:])
                Aunb[1] = Aunb1
                nc.gpsimd.tensor_copy(ATpb[:, ds(0, N)], ATp1[:])
                ATp31 = powp.tile([N, N], F32, tag="ATp31", name="ATp31")

                for lvl in (1, 2, 4, 8, 16):
                    # bf16 branch powers lvl+1 .. 2*lvl-1 (output-only use)
                    for i in range(lvl + 1, 2 * lvl):
                        psq = psp.tile([N, N], F32, tag="ps", bufs=4)
                        nc.tensor.matmul(
                            psq[:], Aunb[lvl][:],
                            ATpb[:, ds((i - lvl - 1) * N, N)],
                            start=True, stop=True)
                        nc.scalar.copy(ATpb[:, ds((i - 1) * N, N)], psq[:])
                        if i == KB - 1:
                            nc.vector.tensor_copy(ATp31[:], psq[:])
                    # fp32 squaring: power 2*lvl
                    psq = psp.tile([N, N], F32, tag="ps", bufs=4)
                    nc.tensor.matmul(psq[:], Aunp[lvl][:], ATp[lvl][:],
                                     start=True, stop=True)
                    t_ = powp.tile([N, N], F32, tag=f"ATp{2 * lvl}",
                                   name=f"ATp{2 * lvl}")
                    nc.vector.tensor_copy(t_[:], psq[:])
                    ATp[2 * lvl] = t_
                    if 2 * lvl <= KB - 1:
                        nc.scalar.copy(ATpb[:, ds((2 * lvl - 1) * N, N)],
                                       psq[:])
                    if 2 * lvl < KB:
                        psu = psp.tile([N, N], F32, tag="ps", bufs=4)
                        nc.tensor.matmul(psu[:], ATp[lvl][:], Aunp[lvl][:],
                                         start=True, stop=True)
                        u_ = powp.tile([N, N], F32, tag=f"Aun{2 * lvl}",
                                       name=f"Aun{2 * lvl}")
                        nc.vector.tensor_copy(u_[:], psu[:])
                        Aunp[2 * lvl] = u_
                        ub = powp.tile([N, N], BF16, tag=f"Aunb{2 * lvl}",
                                       name=f"Aunb{2 * lvl}")
                        nc.scalar.copy(ub[:], psu[:])
                        Aunb[2 * lvl] = ub
                ATp[KB - 1] = ATp31

                # D^T = A~T^K - I
                nc.vector.tensor_sub(DT16[:], ATp[KB][:], ident[:])
                nc.vector.tensor_sub(DT15[:], ATp[KB - 1][:], ident[:])

                # ---- G_i^T = (0.01B)^T A~T^i, i=0..15 ----
                bp_un_b = ep.tile([N, N], BF16, tag="Bpunb")
                nc.vector.tensor_copy(bp_un_b[:], bp_un[:])
                for i in range(KB):
                    psg = psp.tile([N, N], F32, tag="ps", bufs=4,
                                   name=f"G{i}")
                    rhs = identb[:] if i == 0 else ATpb[:, ds((i - 1) * N, N)]
                    nc.tensor.matmul(psg[:], bp_un_b[:], rhs,
                                     start=True, stop=True)
                    if i % 2 == 0:
                        nc.scalar.copy(Gb[:, ds(i * N, N)], psg[:])
                    else:
                        nc.vector.tensor_copy(Gb[:, ds(i * N, N)], psg[:])

                # ---- H_s^T = sum_{i<=s} G_i^T (pairing) ----
                nc.gpsimd.tensor_copy(Hb[:, ds(0, N)], Gb[:, ds(0, N)])
                nc.gpsimd.tensor_add(Hb[:, ds(N, N)], Gb[:, ds(0, N)],
                                     Gb[:, ds(N, N)])
                npair = KB // 2 - 1
                gpair = ep.tile([N, npair * N], BF16, tag="gpair")
                for k in range(1, npair):
                    nc.vector.tensor_add(
                        gpair[:, ds(k * N, N)],
                        Gb[:, ds(2 * k * N, N)],
                        Gb[:, ds((2 * k + 1) * N, N)])
                for k in range(1, KB // 2):
                    # serial chain on odd H; even H branches on Pool
                    nc.gpsimd.tensor_add(
                        Hb[:, ds(2 * k * N, N)],
                        Hb[:, ds((2 * k - 1) * N, N)],
                        Gb[:, ds(2 * k * N, N)])
                    if 2 * k + 1 <= KB - 2:
                        nc.vector.tensor_add(
                            Hb[:, ds((2 * k + 1) * N, N)],
                            Hb[:, ds((2 * k - 1) * N, N)],
                            gpair[:, ds(k * N, N)])

                # ---- c columns: p_s = A~^s bx' then prefix sum ----
                pcols = sp.tile([N, KB], F32, tag="pcols")
                bxp_b = sp.tile([N, BSH], BF16, tag="bxpb")
                nc.vector.tensor_copy(bxp_b[:], bxp_r[:])
                for s in range(KB):
                    lhs = identb[:] if s == 0 else ATpb[:, ds((s - 1) * N, N)]
                    pc = psp.tile([N, N], F32, tag="ps", bufs=4,
                                  name=f"pc{s}")
                    nc.tensor.matmul(pc[:, 0:BSH], lhs,
                                     bxp_b[:], start=True, stop=True)
                    eng = nc.vector if s % 2 == 0 else nc.scalar
                    if s % 2 == 0:
                        nc.vector.tensor_copy(pcols[:, s:s + 1], pc[:, 0:1])
                    else:
                        nc.scalar.copy(pcols[:, s:s + 1], pc[:, 0:1])
                ccols = sp.tile([N, KB], F32, tag="ccols")
                onesc = sp.tile([N, KB], F32, tag="onesc")
                nc.vector.memset(onesc[:], 1.0)
                nc.vector.tensor_tensor_scan(
                    ccols[:], onesc[:], pcols[:], 0.0,
                    op0=ALU.mult, op1=ALU.add,
                )
                # transpose -> Cmat16 [16, 128] bf16
                cpst = psp.tile([KB, N], F32, tag="cs2", bufs=1, name="cpst")
                nc.tensor.transpose(cpst[:], ccols[:], ident[:])
                nc.scalar.copy(Cmat16[:], cpst[:])
                c15ps = psp.tile([1, N], F32, tag="cs3", bufs=2, name="c15ps")
                nc.tensor.transpose(c15ps[:], ccols[:, KB - 1:KB], ident[:])
                nc.scalar.copy(c15row[:], c15ps[:])
                c14ps = psp.tile([1, N], F32, tag="cs3", bufs=2, name="c14ps")
                nc.tensor.transpose(c14ps[:], ccols[:, KB - 2:KB - 1], ident[:])
                nc.scalar.copy(c14row[:], c14ps[:])

            # ------- block loop -------
            with (
                tc.tile_pool(name="xsb", bufs=2) as xsbp,
                tc.tile_pool(name="stg", bufs=2) as stgp,
                tc.tile_pool(name="thb", bufs=2) as thp,
                tc.tile_pool(name="base", bufs=2) as basep,
                tc.tile_pool(name="xps", bufs=1, space="PSUM") as xpsp,
                tc.tile_pool(name="psy", bufs=2, space="PSUM") as psyp,
                tc.tile_pool(name="dps", bufs=1, space="PSUM") as dpsp,
                tc.tile_pool(name="trp", bufs=1, space="PSUM") as trpp,
            ):
                # init: th_init = tanh(C x0 + by), base = x0
                psy0 = psyp.tile([N, BSH], F32, tag="psyq", name="psy0")
                nc.tensor.matmul(psy0[:], CTf32[:], x0_c[:],
                                 start=True, stop=True)
                th_init = thp.tile([N, KB * BSH], BF16, tag="th",
                                   name="th_init")
                nc.scalar.activation(
                    th_init[:, ds((KB - 1) * BSH, BSH)], psy0[:],
                    ACTF.Tanh, bias=by_c[:], scale=1.0
                )
                base_cur = basep.tile([N, BSH], F32, tag="base",
                                      name="base0")
                nc.vector.tensor_copy(base_cur[:], x0_c[:])
                xsb_cur = xsbp.tile([N, KB * BSH], BF16, tag="xsb",
                                    name="xsb0")
                nc.scalar.copy(xsb_cur[:, ds(0, BSH)], x0_c[:])
                th_prev = th_init

                for j in range(NBLK):
                    kb = KB if j < NBLK - 1 else KLAST
                    ncols = (kb - 1) * BSH
                    th1 = th_prev[:, ds((KB - 1) * BSH, BSH)]
                    base_b = xsb_cur[:, ds(0, BSH)]
                    xq = xpsp.tile([N, (KB - 1) * BSH], F32, tag="xq")
                    # consts first (opens accumulation, no deps)
                    for off in range(0, ncols, 512):
                        cwid = min(512, ncols - off)
                        nc.tensor.matmul(
                            xq[:, ds(off, cwid)], Cmat16[:],
                            ind16[:, ds(off, cwid)],
                            start=True, stop=False, skip_group_check=True)
                    for s in range(kb - 1):
                        nc.tensor.matmul(
                            xq[:, ds(s * BSH, BSH)],
                            ATpb[:, ds(s * N, N)], base_b,
                            start=False, stop=False, skip_group_check=True)
                        nc.tensor.matmul(
                            xq[:, ds(s * BSH, BSH)],
                            Hb[:, ds(s * N, N)], th1,
                            start=False, stop=True, skip_group_check=True)
                    # quarters: cast -> psy -> tanh
                    th_cur = thp.tile([N, KB * BSH], BF16, tag="th")
                    nq = KB // 4
                    qbounds = [4 * i for i in range(nq)] + [kb]
                    for q in range(nq):
                        s0, s1 = qbounds[q], qbounds[q + 1]
                        w = (s1 - s0) * BSH
                        cw = min(s1, kb - 1) - s0
                        if cw > 0:
                            nc.vector.tensor_copy(
                                xsb_cur[:, ds((1 + s0) * BSH, cw * BSH)],
                                xq[:, ds(s0 * BSH, cw * BSH)],
                            )
                        psyq = psyp.tile([N, KB // 4 * BSH], F32, tag="psyq")
                        nc.tensor.matmul(
                            psyq[:, ds(0, w)], CTb[:],
                            xsb_cur[:, ds(s0 * BSH, w)],
                            start=True, stop=True)
                        nc.scalar.activation(
                            th_cur[:, ds(s0 * BSH, w)], psyq[:, ds(0, w)],
                            ACTF.Tanh, bias=by_c[:], scale=1.0
                        )
                    # refine: Delta = D x_base + sum G_i th_{kb-1-i} + c_{kb-1}
                    dq = dpsp.tile([N, BSH], F32, tag="dq")
                    nc.tensor.matmul(dq[:], DT16[:] if kb == KB else DT15[:],
                                     base_cur[:], start=True, stop=False,
                                     skip_group_check=True)
                    nc.tensor.matmul(dq[:], (c15row if kb == KB else c14row)[:],
                                     ones64[:], start=False, stop=False,
                                     skip_group_check=True)
                    for s in range(kb - 1, -1, -1):
                        i = kb - 1 - s
                        nc.tensor.matmul(
                            dq[:], Gb[:, ds(i * N, N)],
                            th_cur[:, ds(s * BSH, BSH)],
                            start=False, stop=(s == 0),
                            skip_group_check=True)
                    base_new = basep.tile([N, BSH], F32, tag="base")
                    nc.vector.tensor_add(base_new[:], dq[:], base_cur[:])
                    if j < NBLK - 1:
                        xsb_new = xsbp.tile([N, KB * BSH], BF16, tag="xsb")
                        nc.vector.tensor_add(
                            xsb_new[:, ds(0, BSH)], dq[:], base_cur[:])
                    else:
                        # last block: x_511 goes into this block's col 15
                        xsb_new = None
                        nc.vector.tensor_add(
                            xsb_cur[:, ds((KB - 1) * BSH, BSH)],
                            dq[:], base_cur[:])
                    # output transposes: pairs (2i, 2i+1) of xsb_cur cols
                    t0 = 1 + KB * j
                    for h in range(2):
                        trp = trpp.tile([N, 8 * N], BF16, tag="trp")
                        for i in range(8):
                            nc.tensor.transpose(
                                trp[:, ds(i * N, N)],
                                xsb_cur[:, ds((16 * h + 2 * i) * BSH,
                                              2 * BSH)],
                                identb[:],
                            )
                        stg = stgp.tile([N, 8 * N], BF16, tag="stg")
                        nc.vector.tensor_copy(stg[:], trp[:])
                        nc.sync.dma_start(
                            out=out_e[:, ds((t0 - 1) // 2 + 8 * h, 8),
                                      :].rearrange("b i n -> b (i n)"),
                            in_=stg[0:BSH, :],
                        )
                        nc.sync.dma_start(
                            out=out_o[:, ds((t0 - 1) // 2 + 8 * h, 8),
                                      :].rearrange("b i n -> b (i n)"),
                            in_=stg[BSH:2 * BSH, :],
                        )
                    base_cur = base_new
                    th_prev = th_cur
                    xsb_cur = xsb_new

    nc.compile()
    return nc


_CACHED = {}


def _get_program(*_args, **_kw):
    if "p" not in _CACHED:
        _CACHED["p"] = build_program()
    return _CACHED["p"]


def make_in_maps(inputs):
    X0 = np.ascontiguousarray(np.asarray(inputs["X0"], dtype=np.float32))
    base = {
        name: np.ascontiguousarray(np.asarray(inputs[name], dtype=np.float32))
        for name in PARAM_NAMES
    }
    base["bx"] = np.ascontiguousarray(
        np.asarray(inputs["bx"], dtype=np.float32).reshape(N, 1)
    )
    base["by"] = np.ascontiguousarray(
        np.asarray(inputs["by"], dtype=np.float32).reshape(N, 1)
    )
    in_maps = []
    for c in range(NCORES):
        m = dict(base)
        m["x0"] = np.ascontiguousarray(X0[c * BSH:(c + 1) * BSH].T)
        in_maps.append(m)
    return in_maps


def run_spmd(inputs, *_args, trace=False, tmpdir=None, **_kw):
    nc = _get_program()
    in_maps = make_in_maps(inputs)
    res = run_bass_kernel_spmd(
        nc, in_maps, list(range(NCORES)), trace=trace, tmpdir=tmpdir
    )
    outs = []
    for c in range(NCORES):
        ev = np.asarray(res.results[c]["out_e"]).astype(np.float32)
        od = np.asarray(res.results[c]["out_o"]).astype(np.float32)
        full = np.empty((BSH, TMAX, N), dtype=np.float32)
        full[:, 0::2, :] = ev
        full[:, 1::2, :] = od
        outs.append(full)
    return np.concatenate(outs, axis=0), res


def kernel(**inputs):
    full, _ = run_spmd(inputs)
    return full
